# revision 27
# baseline (speedup 1.0000x reference)
"""GQA kernel for Trainium2, 8 NeuronCores — mixed bf16/fp8-DoubleRow.

Sharding: core c -> batch b = c//4, kv-head-group g = c%4.
Each core: 1 batch, 2 KV heads (2g, 2g+1), 8 Q heads, row-shard of W_o
(rows 512g..512g+512). Host sums the 4 partials per batch, /1024, + bo.

Precision plan (gate: rel err < 2e-2 vs abs-max):
  - Q/K/V projections: 3-term error-compensated fp8 DoubleRow,
      P = x8@W8  (psum1)   +   [(32dx)8@W8 + x8@(32dW)8]/32  (psum2),
    residuals prepped on host, merged by DVE into bf16 (error ~0.1%).
    Weights carry a x32 base scale for fp8 range; Q/K/V come out x32.
  - Scores + A@V for q-tiles 1..3 (n_eff > 512): fp8 DoubleRow from
    fp8 copies of Q/K/at/V — softmax normalization suppresses these
    errors by ~1/sqrt(n_eff).
  - Scores + A@V for q-tile 0 and the O projection: bf16 (quantization
    there is not normalization-suppressed).
  - exp bias -3.3 keeps fp8 'at' in [3e-8, 81] (e4m3 max 240).
  - Causal masks: PE matmuls with +-240 fp8 constants (2-plane product
    -115200 -> exp ~ 0).

Streams (h, kv, qg): key-block PAIRS fill a [128,2,512] psum group
(2-bank, ping-pong) -> one wide exp on ACT into at2 ring buffers.
Diagonal quads split {j0,j1} (full width) / {j2,j3} (cols 256:512,
rects memset 0).  A@V runs lagged one stream from saved at2: 4 q-subtile
slots sequentially through the work-psum ring, DVE recip+mul -> an2
(bf16), PE transpose -> attnT for the bf16 O projection.  O-proj psum
chunks DMA straight to DRAM as f32 (host unscales 1/1024).
Projection/O chunks stream between attention pairs as budgeted fills.

PSUM: 4 banks score ping-pong + 4-bank work ring.
"""

import numpy as np

E = 2048
S = 2048
B = 2
D = 64
NCORE = 8
NKB = S // 128      # 16 key blocks
WS = 32.0           # host base scale for all weights

_CACHE = {}
# tile jb holds q-heads (jb, jb+4): kv0 head dims at partitions 0:64,
# kv1 head dims at 64:128, matching the K/V partition layout
HEAD_PERM = [0, 4, 1, 5, 2, 6, 3, 7]


def _build():
    import concourse.bass as bass
    import concourse.tile as tile
    from concourse import mybir
    from concourse import bacc
    from concourse.masks import make_identity

    F32 = mybir.dt.float32
    BF16 = mybir.dt.bfloat16
    FP8 = mybir.dt.float8e4
    Exp = mybir.ActivationFunctionType.Exp
    DR = mybir.MatmulPerfMode.DoubleRow
    ADD = mybir.AluOpType.add
    MULT = mybir.AluOpType.mult

    nc = bacc.Bacc("TRN2", target_bir_lowering=False, debug=False,
                   num_devices=NCORE)

    XT = nc.declare_dram_parameter("xt", [128, 8, 2, S], FP8, isOutput=False)
    XR = nc.declare_dram_parameter("xr", [128, 8, 2, S], FP8, isOutput=False)
    WQ = nc.declare_dram_parameter("wq", [128, 4, 8, 2, 128], FP8,
                                   isOutput=False)
    WQR = nc.declare_dram_parameter("wqr", [128, 4, 8, 2, 128], FP8,
                                    isOutput=False)
    WKV = nc.declare_dram_parameter("wkv", [128, 4, 8, 2, 128], FP8,
                                    isOutput=False)
    WO8 = nc.declare_dram_parameter("wo8", [128, 2, 2, E], FP8,
                                    isOutput=False)
    DWO = nc.declare_dram_parameter("dwo", [128, 2, 2, E], FP8,
                                    isOutput=False)
    BIAS = nc.declare_dram_parameter("bias", [128, 8], F32, isOutput=False)
    MSK = nc.declare_dram_parameter("msk", [128, 896], FP8, isOutput=False)
    OUT = nc.declare_dram_parameter("out", [S, E], BF16, isOutput=True)

    with tile.TileContext(nc) as tc:
        with tc.tile_pool(name="persist", bufs=1) as persist, \
             tc.tile_pool(name="atf", bufs=2) as atf, \
             tc.tile_pool(name="atb", bufs=2) as atb, \
             tc.tile_pool(name="anp", bufs=2) as anp, \
             tc.tile_pool(name="vsp", bufs=2) as vsp, \
             tc.tile_pool(name="rlp", bufs=4) as rlp, \
             tc.tile_pool(name="tmp", bufs=2) as tmpp, \
             tc.tile_pool(name="osp", bufs=2) as osp, \
             tc.tile_pool(name="antp", bufs=4) as antp, \
             tc.tile_pool(name="scp", bufs=2, space="PSUM") as scp, \
             tc.tile_pool(name="wkp", bufs=4, space="PSUM") as wkp:

            # ---- persistent SBUF ----
            xt = persist.tile([128, 8, 2, S], FP8, tag="xt")
            xr = persist.tile([128, 8, 2, S], FP8, tag="xr")
            wq = persist.tile([128, 4, 8, 2, 128], FP8, tag="wq")
            wqr = persist.tile([128, 4, 8, 2, 128], FP8, tag="wqr")
            wkv = persist.tile([128, 4, 8, 2, 128], FP8, tag="wkv")
            wk, wkr, wv, wvr = (wkv[:, 0], wkv[:, 1], wkv[:, 2],
                                wkv[:, 3])
            wo8 = persist.tile([128, 2, 2, E], FP8, tag="wo8")
            dwo = persist.tile([128, 2, 2, E], FP8, tag="dwo")
            qtb = [persist.tile([128, S], BF16, tag=f"qtb{j}", name=f"qtb{j}")
                   for j in range(4)]
            qt8 = [persist.tile([128, S], FP8, tag=f"qt8{j}", name=f"qt8{j}")
                   for j in range(4)]
            ktb = persist.tile([128, S], BF16, tag="ktb")
            kt8 = persist.tile([128, 2, S], FP8, tag="kt8")
            vtb = persist.tile([128, 2, NKB, D + 1], BF16, tag="vtb")
            vt8 = persist.tile([128, 2, NKB, D + 1], FP8, tag="vt8")
            attnT8 = [persist.tile([128, 2, S], FP8, tag=f"attnT8{j}",
                                   name=f"attnT8{j}") for j in range(2)]
            datT8 = [persist.tile([128, 2, S], FP8, tag=f"datT8{j}",
                                  name=f"datT8{j}") for j in range(2)]
            msk = persist.tile([128, 896], FP8, tag="msk")
            cmt = msk[:, 0:256].rearrange("p (a b) -> p a b", a=2)
            cmx = msk[:, 256:512].rearrange("p (a b) -> p a b", a=2)
            seli = msk[:, 512:640]
            selw = msk[:, 640:896]
            id128 = persist.tile([128, 128], BF16, tag="id128")
            id64 = persist.tile([128, D], BF16, tag="id64")
            bias_t = persist.tile([128, 8], F32, tag="bias")

            # ---- input DMAs, ordered so first streams start early ----
            nc.sync.dma_start(out=wq[:, 0], in_=WQ[:, 0])
            nc.sync.dma_start(out=xt[:, :, :, 0:512], in_=XT[:, :, :, 0:512])
            nc.sync.dma_start(out=wkv[:, 0:2], in_=WKV[:, 0:2])
            nc.sync.dma_start(out=xr[:, :, :, 0:512], in_=XR[:, :, :, 0:512])
            nc.sync.dma_start(out=wqr[:, 0], in_=WQR[:, 0])
            nc.sync.dma_start(out=bias_t, in_=BIAS[:, :])
            nc.sync.dma_start(out=msk, in_=MSK[:, :])
            nc.sync.dma_start(out=wkv[:, 2:4], in_=WKV[:, 2:4])
            nc.sync.dma_start(out=wq[:, 1], in_=WQ[:, 1])
            nc.sync.dma_start(out=wqr[:, 1], in_=WQR[:, 1])
            nc.sync.dma_start(out=xt[:, :, :, 512:1024],
                              in_=XT[:, :, :, 512:1024])
            nc.sync.dma_start(out=xr[:, :, :, 512:1024],
                              in_=XR[:, :, :, 512:1024])
            nc.sync.dma_start(out=wq[:, 2], in_=WQ[:, 2])
            nc.sync.dma_start(out=wq[:, 3], in_=WQ[:, 3])
            nc.sync.dma_start(out=wqr[:, 2], in_=WQR[:, 2])
            nc.sync.dma_start(out=wqr[:, 3], in_=WQR[:, 3])
            nc.sync.dma_start(out=xt[:, :, :, 1024:1536],
                              in_=XT[:, :, :, 1024:1536])
            nc.sync.dma_start(out=xt[:, :, :, 1536:2048],
                              in_=XT[:, :, :, 1536:2048])
            nc.sync.dma_start(out=xr[:, :, :, 1024:1536],
                              in_=XR[:, :, :, 1024:1536])
            nc.sync.dma_start(out=xr[:, :, :, 1536:2048],
                              in_=XR[:, :, :, 1536:2048])
            nc.sync.dma_start(out=wo8, in_=WO8[:, :])
            nc.sync.dma_start(out=dwo, in_=DWO[:, :])

            make_identity(nc, id128)
            make_identity(nc, id64[0:64, :])
            make_identity(nc, id64[64:128, :])
            nc.gpsimd.memset(kt8[:, 1, :], 0.0)        # zero K plane
            nc.gpsimd.memset(vtb[:, :, :, D:D + 1], 1.0)   # denom ones
            nc.gpsimd.memset(vt8[:, :, :, D:D + 1], 1.0)

            # ---------------- comp3 projection fills ----------------
            def comp3(st, w8, wr, cols, n):
                """3-term compensated projection as 3 PE sub-units."""
                def u1():
                    st["ps1"] = wkp.tile([128, n], F32, tag="wk", name="ps1")
                    for c2 in range(8):
                        nc.tensor.matmul(
                            st["ps1"], w8[:, c2], xt[:, c2, :, cols],
                            start=(c2 == 0), stop=(c2 == 7),
                            perf_mode=DR, skip_group_check=True)

                def u2():
                    st["ps2"] = wkp.tile([128, n], F32, tag="wk", name="ps2")
                    for c2 in range(8):
                        nc.tensor.matmul(
                            st["ps2"], w8[:, c2], xr[:, c2, :, cols],
                            start=(c2 == 0), stop=False,
                            perf_mode=DR, skip_group_check=True)

                def u3():
                    for c2 in range(8):
                        nc.tensor.matmul(
                            st["ps2"], wr[:, c2], xt[:, c2, :, cols],
                            start=False, stop=(c2 == 7),
                            perf_mode=DR, skip_group_check=True)
                return [u1, u2, u3]

            def merge(st, out_b, bcol, n):
                """DVE: out_b = ps1 + ps2/32 + bias."""
                tm = tmpp.tile([128, 512], F32, tag="tm", name="tm")
                nc.vector.tensor_scalar(
                    tm[:, 0:n], st["ps2"], 1.0 / 32.0,
                    bias_t[:, bcol:bcol + 1], MULT, ADD)
                nc.vector.tensor_add(out_b, st["ps1"], tm[:, 0:n])

            def make_qproj(t5, jb):
                cols = bass.ds(t5 * 512, 512)
                st = {}

                def seg(which, c0, alloc=False, start=False, stop=False):
                    def f():
                        if alloc:
                            st[which] = wkp.tile([128, 512], F32, tag="wk",
                                                 name=which)
                        w = wq[:, jb] if which == "ps1" or c0 >= 100                             else wq[:, jb]
                        for c2 in range(c0 % 100, c0 % 100 + 4):
                            if which == "ps1":
                                lhs, rhs = wq[:, jb, c2], xt[:, c2, :, cols]
                            elif c0 < 100:
                                lhs, rhs = wq[:, jb, c2], xr[:, c2, :, cols]
                            else:
                                lhs, rhs = wqr[:, jb, c2], xt[:, c2, :, cols]
                            nc.tensor.matmul(
                                st[which], lhs, rhs,
                                start=(start and c2 == c0 % 100),
                                stop=(stop and c2 == c0 % 100 + 3),
                                perf_mode=DR, skip_group_check=True)
                    return f

                def fin():
                    seg("ps2", 104, stop=True)()
                    merge(st, qtb[jb][:, cols], jb, 512)
                    nc.gpsimd.tensor_copy(qt8[jb][:, cols], qtb[jb][:, cols])
                return [seg("ps1", 0, alloc=True, start=True),
                        seg("ps1", 4, stop=True),
                        seg("ps2", 0, alloc=True, start=True),
                        seg("ps2", 4),
                        seg("ps2", 100),
                        fin]

            def make_kproj(tg):
                cols = bass.ds(tg * 256, 256)
                st = {}
                us = comp3(st, wk, wkr, cols, 256)

                def fin():
                    us[2]()
                    merge(st, ktb[:, cols], 4, 256)
                    nc.gpsimd.tensor_copy(kt8[:, 0, cols], ktb[:, cols])
                return [us[0], us[1], fin]

            def make_vproj(tg):
                cols = bass.ds(tg * 256, 256)
                st = {}
                us = comp3(st, wv, wvr, cols, 256)

                def fin():
                    us[2]()
                    st["vs"] = vsp.tile([128, 256], BF16, tag="vs",
                                        name="vs")
                    merge(st, st["vs"], 5, 256)

                def transp(kv):
                    def f():
                        tp = wkp.tile([128, 2, D], BF16, tag="wk",
                                      name="vtp")
                        for tc2 in range(2):
                            nc.tensor.transpose(
                                tp[:, tc2, :],
                                st["vs"][kv * 64:kv * 64 + 64,
                                         tc2 * 128:(tc2 + 1) * 128],
                                id64[kv * 64:kv * 64 + 64, :])
                        nc.vector.tensor_copy(
                            vtb[:, kv, 2 * tg:2 * tg + 2, 0:D], tp)
                        nc.gpsimd.tensor_copy(
                            vt8[:, kv, 2 * tg:2 * tg + 2, 0:D],
                            vtb[:, kv, 2 * tg:2 * tg + 2, 0:D])
                    return f
                return [us[0], us[1], fin, transp(0), transp(1)]

            osp_tiles = {}
            Copy = mybir.ActivationFunctionType.Copy

            def emit_oproj_chunk(tb, ng, on_act=False):
                if tb not in osp_tiles:
                    osp_tiles[tb] = osp.tile([128, E], BF16, tag="os",
                                             name="ostage")
                op = wkp.tile([128, 512], F32, tag="wk", name="opc")
                tc_ = bass.ds(tb * 128, 128)
                ngc = bass.ds(ng * 512, 512)
                for ti, (lh, rh) in enumerate(
                        [(attnT8, wo8), (datT8, wo8), (attnT8, dwo)]):
                    for jbp in range(2):
                        nc.tensor.matmul(
                            op, lh[jbp][:, :, tc_], rh[:, jbp, :, ngc],
                            start=(ti == 0 and jbp == 0),
                            stop=(ti == 2 and jbp == 1),
                            perf_mode=DR, skip_group_check=True)
                if on_act:
                    nc.scalar.activation(
                        osp_tiles[tb][:, ng * 512:(ng + 1) * 512], op,
                        Copy, bias=0.0, scale=bias_t[:, 6:7])
                else:
                    nc.vector.tensor_scalar_mul(
                        osp_tiles[tb][:, ng * 512:(ng + 1) * 512], op,
                        bias_t[:, 6:7])
                if ng == 3:
                    nc.sync.dma_start(out=OUT[tb * 128:(tb + 1) * 128, :],
                                      in_=osp_tiles[tb])

            def qu(t5, jb):
                return make_qproj(t5, jb)

            def kvu(tg):
                return make_kproj(tg) + make_vproj(tg)

            op_st = {}

            def emit_oproj_half(tb, ng, half):
                key = (tb, ng)
                if half == 0:
                    if tb not in osp_tiles:
                        osp_tiles[tb] = osp.tile([128, E], BF16, tag="os",
                                                 name="ostage")
                    op_st[key] = wkp.tile([128, 512], F32, tag="wk",
                                          name="opc")
                op = op_st[key]
                tc_ = bass.ds(tb * 128, 128)
                ngc = bass.ds(ng * 512, 512)
                terms = [(attnT8, wo8, 0), (attnT8, wo8, 1),
                         (datT8, wo8, 0), (datT8, wo8, 1),
                         (attnT8, dwo, 0), (attnT8, dwo, 1)]
                rng = terms[0:3] if half == 0 else terms[3:6]
                for ti, (lh, rh, jbp) in enumerate(rng):
                    nc.tensor.matmul(
                        op, lh[jbp][:, :, tc_], rh[:, jbp, :, ngc],
                        start=(half == 0 and ti == 0),
                        stop=(half == 1 and ti == 2),
                        perf_mode=DR, skip_group_check=True)
                if half == 1:
                    nc.vector.tensor_scalar_mul(
                        osp_tiles[tb][:, ng * 512:(ng + 1) * 512], op,
                        bias_t[:, 6:7])
                    if ng == 3:
                        nc.sync.dma_start(
                            out=OUT[tb * 128:(tb + 1) * 128, :],
                            in_=osp_tiles[tb])
                    del op_st[key]

            def ou(tbs):
                return [(lambda tb=tb, ng=ng, hf=hf:
                         emit_oproj_half(tb, ng, hf))
                        for tb in tbs for ng in range(4) for hf in (0, 1)]

            # ---------------- attention streams ----------------
            def emit_stream_scores(at_t, qg, h, kv):
                nkb = 4 * qg + 4
                npair = nkb // 2
                qoff = kv * 64
                q0 = qg * 512

                def score8(bank_ap, kb, c0, start, stop):
                    n = 512 - c0
                    nc.tensor.matmul(
                        bank_ap[:, c0:512],
                        kt8[qoff:qoff + 64, :, kb * 128:(kb + 1) * 128],
                        qt8[h][qoff:qoff + 64, q0 + c0:q0 + 512]
                        .unsqueeze(1).broadcast_to([64, 2, n]),
                        start=start, stop=stop,
                        perf_mode=DR, skip_group_check=True)

                def scoreb(bank_ap, kb, c0, start, stop):
                    nc.tensor.matmul(
                        bank_ap[:, c0:512],
                        ktb[qoff:qoff + 64, kb * 128:(kb + 1) * 128],
                        qtb[h][qoff:qoff + 64, q0 + c0:q0 + 512],
                        start=start, stop=stop, skip_group_check=True)

                score = scoreb if qg == 0 else score8

                def mask_tril(bank_ap, c0):
                    nc.tensor.matmul(
                        bank_ap[:, c0:c0 + 128], cmt,
                        seli.unsqueeze(1).broadcast_to([128, 2, 128]),
                        start=True, stop=False,
                        perf_mode=DR, skip_group_check=True)

                for pr in range(npair):
                    kb0 = 2 * pr
                    di = pr - (npair - 2)  # 0 => {j0,j1}, 1 => {j2,j3}
                    sc = scp.tile([128, 2, 512], F32, tag="sc", name="sc")
                    if di < 0:
                        for b in range(2):
                            score(sc[:, b], kb0 + b, 0, True, True)
                    elif di == 0:
                        mask_tril(sc[:, 0], 0)
                        score(sc[:, 0], kb0, 0, False, True)
                        nc.tensor.matmul(
                            sc[:, 1, 0:256], cmx,
                            selw.unsqueeze(1).broadcast_to([128, 2, 256]),
                            start=True, stop=False,
                            perf_mode=DR, skip_group_check=True)
                        score(sc[:, 1], kb0 + 1, 128, False, True)
                    else:
                        mask_tril(sc[:, 0], 256)
                        score(sc[:, 0], kb0, 256, False, True)
                        # j3: cols [256:384) fully masked + tril [384:512)
                        nc.tensor.matmul(
                            sc[:, 1, 256:512], cmx,
                            selw.unsqueeze(1).broadcast_to([128, 2, 256]),
                            start=True, stop=False,
                            perf_mode=DR, skip_group_check=True)
                        score(sc[:, 1], kb0 + 1, 384, False, True)
                    if di == 1:
                        nc.scalar.activation(
                            at_t[:, kb0:kb0 + 2, 256:512],
                            sc[:, :, 256:512], Exp,
                            bias=bias_t[:, 7:8], scale=1.0 / 8192.0)
                    else:
                        nc.scalar.activation(
                            at_t[:, kb0:kb0 + 2, :], sc, Exp,
                            bias=bias_t[:, 7:8], scale=1.0 / 8192.0)
                    yield

            def emit_avburst(at_t, qg, h, kv, an_t):
                nkb = 4 * qg + 4
                npair = nkb // 2
                for s in range(4):
                    slot = wkp.tile([128, D + 1], F32, tag="wk", name="av")
                    if qg == 0:
                        for kb in range(nkb):
                            nc.tensor.matmul(
                                slot,
                                at_t[:, kb, s * 128:(s + 1) * 128],
                                vtb[:, kv, kb, 0:D + 1],
                                start=(kb == 0), stop=(kb == nkb - 1),
                                skip_group_check=True)
                    else:
                        for pr in range(npair):
                            nc.tensor.matmul(
                                slot,
                                at_t[:, 2 * pr:2 * pr + 2,
                                     s * 128:(s + 1) * 128],
                                vt8[:, kv, 2 * pr:2 * pr + 2, 0:D + 1],
                                start=(pr == 0), stop=(pr == npair - 1),
                                perf_mode=DR, skip_group_check=True)
                    rl = rlp.tile([128, 1], F32, tag="rl", name="rl")
                    nc.vector.reciprocal_approx_fast(rl, slot[:, D:D + 1])
                    nc.vector.tensor_scalar_mul(
                        an_t[:, s, kv * 64:kv * 64 + 64], slot[:, 0:D],
                        rl[:, 0:1])

            def emit_antranspose(an_t, qg, h):
                q0 = qg * 512
                jbp, pl = h // 2, h % 2
                for qb in range(4):
                    tp = wkp.tile([128, 128], BF16, tag="wk", name="atp2")
                    nc.tensor.transpose(tp, an_t[:, qb, :], id128)
                    cols = bass.ds(q0 + qb * 128, 128)
                    ab = antp.tile([128, 128], BF16, tag="ab", name="ab")
                    nc.vector.tensor_copy(ab, tp)
                    nc.gpsimd.tensor_copy(attnT8[jbp][:, pl, cols], ab)
                    nc.gpsimd.tensor_sub(datT8[jbp][:, pl, cols], ab,
                                         attnT8[jbp][:, pl, cols])

            # ---------------- phase 1: first-stream prereqs ----------------
            for f in qu(0, 0) + kvu(0) + kvu(1):
                f()

            fills = {
                0: qu(0, 1) + qu(0, 2) + qu(0, 3) + qu(1, 0) + qu(1, 1),
                1: qu(1, 2) + qu(1, 3) + qu(2, 0) + qu(2, 1) +
                   kvu(4) + kvu(5),
                2: qu(2, 2) + qu(2, 3) + qu(3, 0) + qu(3, 1) +
                   kvu(6) + kvu(7) + ou((0, 1)),
                3: qu(3, 2) + qu(3, 3) + ou((2, 3)) + ou((4, 5)) +
                   ou((6, 7)) + ou((8, 9, 10, 11)),
            }

            # ---------------- main stream loop ----------------
            prev = None          # (at_t, qg, h, kv, an_t)
            an_cur = {}
            pend_tr = []

            pre_qg = {1: kvu(2) + kvu(3)}

            for qg in range(4):
                for f in pre_qg.get(qg, []):
                    f()
                units = list(fills[qg])
                nu = len(units)
                done = 0
                npair = 2 * qg + 2
                nstep = 8 * npair
                for si, (h, kv) in enumerate([(h, kv) for h in range(4)
                                              for kv in range(2)]):
                    nkb = 4 * qg + 4
                    if qg == 0:
                        at_t = atb.tile([128, 4, 512], BF16, tag="atb",
                                        name="atb")
                    else:
                        at_t = atf.tile([128, NKB, 512], FP8, tag="atf",
                                        name="atf")
                    # zero never-exp'd rects of the {j2,j3} diag pair
                    nc.gpsimd.memset(at_t[:, nkb - 2:nkb, 0:256], 0.0)
                    if kv == 0:
                        an_cur[h] = anp.tile([128, 4, 128], BF16, tag="an",
                                             name="an2")
                    gen = emit_stream_scores(at_t, qg, h, kv)
                    step = 0
                    for _ in gen:
                        step += 1
                        # lagged work tucked behind this stream's first exps
                        if step == 1 and prev is not None:
                            emit_avburst(*prev)
                            if prev[3] == 1:
                                pend_tr.append((prev[4], prev[1], prev[2]))
                            prev = None
                        if step == 2:
                            while len(pend_tr) > 1:
                                a, g, hh = pend_tr.pop(0)
                                emit_antranspose(a, g, hh)
                        want = nu * (si * npair + step) // nstep
                        while done < want:
                            units[done]()
                            done += 1
                    prev = (at_t, qg, h, kv, an_cur[h])
                while done < nu:
                    units[done]()
                    done += 1

            # ---------------- tail ----------------
            emit_avburst(*prev)
            pend_tr.append((prev[4], prev[1], prev[2]))
            while pend_tr:
                a, g, hh = pend_tr.pop(0)
                emit_antranspose(a, g, hh)
            for tb in (12, 13, 14, 15):
                for ng in range(4):
                    emit_oproj_chunk(tb, ng, on_act=(ng % 2 == 1))

    nc.compile()
    return nc


def _prep_core_inputs(c, x, Wq, bq, Wk, bk, Wv, bv, Wo, xt_cache, fp8):
    import ml_dtypes
    bf16 = ml_dtypes.bfloat16
    g = c % 4
    b = c // 4
    f32 = np.float32
    if b not in xt_cache:
        # xt[p, c2, i, t] = x[b, t, 128*(2*c2+i)+p]; xr = 32*residual
        xm = np.ascontiguousarray(
            x[b].T.reshape(16, 128, S).reshape(8, 2, 128, S)
            .transpose(2, 0, 1, 3))
        x8 = xm.astype(fp8)
        xrr = ((xm - x8.astype(f32)) * 32.0).astype(fp8)
        xt_cache[b] = (x8, xrr)
    x8, xrr = xt_cache[b]

    def split8(wm):
        w8 = wm.astype(fp8)
        wr = ((wm - w8.astype(f32)) * 32.0).astype(fp8)
        return w8, wr

    wq_s = Wq[:, 512 * g:512 * (g + 1)].reshape(E, 8, 64)
    wq_s = wq_s[:, HEAD_PERM, :].reshape(E, 512) * f32(WS)
    wq_m = np.ascontiguousarray(
        wq_s.reshape(8, 2, 128, 4, 128).transpose(2, 3, 0, 1, 4))
    wq8, wqr = split8(wq_m)
    wk_s = Wk[:, 128 * g:128 * (g + 1)] * f32(WS)
    wk_m = np.ascontiguousarray(
        wk_s.reshape(8, 2, 128, 128).transpose(2, 0, 1, 3))
    wk8, wkr = split8(wk_m)
    wv_s = Wv[:, 128 * g:128 * (g + 1)] * f32(WS)
    wv_m = np.ascontiguousarray(
        wv_s.reshape(8, 2, 128, 128).transpose(2, 0, 1, 3))
    wv8, wvr = split8(wv_m)
    wo_s = Wo[512 * g:512 * (g + 1), :].reshape(8, 64, E)
    wo_s = wo_s[HEAD_PERM, :, :].reshape(512, E) * f32(WS)
    wo_m = np.ascontiguousarray(
        wo_s.reshape(2, 2, 128, E).transpose(2, 0, 1, 3))
    wo8 = wo_m.astype(fp8)
    dwo = (wo_m - wo8.astype(f32)).astype(fp8)
    bias = np.zeros((128, 8), f32)
    bq_s = bq[512 * g:512 * (g + 1)].reshape(8, 64)[HEAD_PERM, :].reshape(512)
    bias[:, 0:4] = bq_s.reshape(4, 128).T * WS
    bias[:, 4] = bk[128 * g:128 * (g + 1)] * WS
    bias[:, 5] = bv[128 * g:128 * (g + 1)] * WS
    bias[:, 6] = 1.0 / 1024.0
    bias[:, 7] = -3.3
    pp = np.arange(128)[:, None]
    kk = np.arange(128)[None, :]
    cmt = np.repeat(np.where(pp < kk, -240.0, 0.0)[:, None, :],
                    2, axis=1).astype(fp8)
    cmx = np.repeat(np.where(pp <= kk, -240.0, 0.0)[:, None, :],
                    2, axis=1).astype(fp8)
    seli = (np.eye(128, dtype=f32) * 240.0).astype(fp8)
    selw = np.zeros((128, 256), f32)
    selw[0, 0:128] = 240.0
    for cc in range(128, 255):
        selw[cc - 127, cc] = 240.0
    selw = selw.astype(fp8)
    wkv = np.stack([wk8, wkr, wv8, wvr], axis=1)
    msk = np.concatenate(
        [cmt.reshape(128, 256), cmx.reshape(128, 256), seli, selw],
        axis=1)
    return {"xt": x8, "xr": xrr, "wq": wq8, "wqr": wqr, "wkv": wkv,
            "wo8": wo8, "dwo": dwo, "bias": bias, "msk": msk}


def kernel(**inputs):
    import ml_dtypes
    from concourse.bass_utils import run_bass_kernel_spmd

    fp8 = ml_dtypes.float8_e4m3
    x = np.asarray(inputs["x"], np.float32)
    Wq = np.asarray(inputs["Wq"], np.float32)
    bq = np.asarray(inputs["bq"], np.float32)
    Wk = np.asarray(inputs["Wk"], np.float32)
    bk = np.asarray(inputs["bk"], np.float32)
    Wv = np.asarray(inputs["Wv"], np.float32)
    bv = np.asarray(inputs["bv"], np.float32)
    Wo = np.asarray(inputs["Wo"], np.float32)
    bo = np.asarray(inputs["bo"], np.float32)

    if "nc" not in _CACHE:
        _CACHE["nc"] = _build()
    nc = _CACHE["nc"]

    xt_cache = {}
    in_maps = [_prep_core_inputs(c, x, Wq, bq, Wk, bk, Wv, bv, Wo,
                                 xt_cache, fp8)
               for c in range(NCORE)]
    res = run_bass_kernel_spmd(nc, in_maps, list(range(NCORE)))
    parts = [res.results[c]["out"].astype(np.float32) for c in range(NCORE)]
    out0 = parts[0] + parts[1] + parts[2] + parts[3] + bo
    out1 = parts[4] + parts[5] + parts[6] + parts[7] + bo
    return np.stack([out0, out1]).astype(np.float32)


# revision 28
# speedup vs baseline: 1.0035x; 1.0035x over previous
"""GQA kernel for Trainium2, 8 NeuronCores — mixed bf16/fp8-DoubleRow.

Sharding: core c -> batch b = c//4, kv-head-group g = c%4.
Each core: 1 batch, 2 KV heads (2g, 2g+1), 8 Q heads, row-shard of W_o
(rows 512g..512g+512). Host sums the 4 partials per batch, /1024, + bo.

Precision plan (gate: rel err < 2e-2 vs abs-max):
  - Q/K/V projections: 3-term error-compensated fp8 DoubleRow,
      P = x8@W8  (psum1)   +   [(32dx)8@W8 + x8@(32dW)8]/32  (psum2),
    residuals prepped on host, merged by DVE into bf16 (error ~0.1%).
    Weights carry a x32 base scale for fp8 range; Q/K/V come out x32.
  - Scores + A@V for q-tiles 1..3 (n_eff > 512): fp8 DoubleRow from
    fp8 copies of Q/K/at/V — softmax normalization suppresses these
    errors by ~1/sqrt(n_eff).
  - Scores + A@V for q-tile 0 and the O projection: bf16 (quantization
    there is not normalization-suppressed).
  - exp bias -3.3 keeps fp8 'at' in [3e-8, 81] (e4m3 max 240).
  - Causal masks: PE matmuls with +-240 fp8 constants (2-plane product
    -115200 -> exp ~ 0).

Streams (h, kv, qg): key-block PAIRS fill a [128,2,512] psum group
(2-bank, ping-pong) -> one wide exp on ACT into at2 ring buffers.
Diagonal quads split {j0,j1} (full width) / {j2,j3} (cols 256:512,
rects memset 0).  A@V runs lagged one stream from saved at2: 4 q-subtile
slots sequentially through the work-psum ring, DVE recip+mul -> an2
(bf16), PE transpose -> attnT for the bf16 O projection.  O-proj psum
chunks DMA straight to DRAM as f32 (host unscales 1/1024).
Projection/O chunks stream between attention pairs as budgeted fills.

PSUM: 4 banks score ping-pong + 4-bank work ring.
"""

import numpy as np

E = 2048
S = 2048
B = 2
D = 64
NCORE = 8
NKB = S // 128      # 16 key blocks
WS = 32.0           # host base scale for all weights

_CACHE = {}
# tile jb holds q-heads (jb, jb+4): kv0 head dims at partitions 0:64,
# kv1 head dims at 64:128, matching the K/V partition layout
HEAD_PERM = [0, 4, 1, 5, 2, 6, 3, 7]


def _build():
    import concourse.bass as bass
    import concourse.tile as tile
    from concourse import mybir
    from concourse import bacc
    from concourse.masks import make_identity

    F32 = mybir.dt.float32
    BF16 = mybir.dt.bfloat16
    FP8 = mybir.dt.float8e4
    Exp = mybir.ActivationFunctionType.Exp
    DR = mybir.MatmulPerfMode.DoubleRow
    ADD = mybir.AluOpType.add
    MULT = mybir.AluOpType.mult

    nc = bacc.Bacc("TRN2", target_bir_lowering=False, debug=False,
                   num_devices=NCORE)

    XT = nc.declare_dram_parameter("xt", [128, 8, 2, S], FP8, isOutput=False)
    XR = nc.declare_dram_parameter("xr", [128, 8, 2, S], FP8, isOutput=False)
    WQ = nc.declare_dram_parameter("wq", [128, 4, 8, 2, 128], FP8,
                                   isOutput=False)
    WQR = nc.declare_dram_parameter("wqr", [128, 4, 8, 2, 128], FP8,
                                    isOutput=False)
    WKV = nc.declare_dram_parameter("wkv", [128, 4, 8, 2, 128], FP8,
                                    isOutput=False)
    WO8 = nc.declare_dram_parameter("wo8", [128, 2, 2, E], FP8,
                                    isOutput=False)
    DWO = nc.declare_dram_parameter("dwo", [128, 2, 2, E], FP8,
                                    isOutput=False)
    BIAS = nc.declare_dram_parameter("bias", [128, 8], F32, isOutput=False)
    MSK = nc.declare_dram_parameter("msk", [128, 896], FP8, isOutput=False)
    OUT = nc.declare_dram_parameter("out", [S, E], BF16, isOutput=True)

    with tile.TileContext(nc) as tc:
        with tc.tile_pool(name="persist", bufs=1) as persist, \
             tc.tile_pool(name="atf", bufs=2) as atf, \
             tc.tile_pool(name="atb", bufs=2) as atb, \
             tc.tile_pool(name="anp", bufs=2) as anp, \
             tc.tile_pool(name="vsp", bufs=2) as vsp, \
             tc.tile_pool(name="rlp", bufs=4) as rlp, \
             tc.tile_pool(name="tmp", bufs=2) as tmpp, \
             tc.tile_pool(name="osp", bufs=2) as osp, \
             tc.tile_pool(name="antp", bufs=4) as antp, \
             tc.tile_pool(name="scp", bufs=2, space="PSUM") as scp, \
             tc.tile_pool(name="wkp", bufs=4, space="PSUM") as wkp:

            # ---- persistent SBUF ----
            xt = persist.tile([128, 8, 2, S], FP8, tag="xt")
            xr = persist.tile([128, 8, 2, S], FP8, tag="xr")
            wq = persist.tile([128, 4, 8, 2, 128], FP8, tag="wq")
            wqr = persist.tile([128, 4, 8, 2, 128], FP8, tag="wqr")
            wkv = persist.tile([128, 4, 8, 2, 128], FP8, tag="wkv")
            wk, wkr, wv, wvr = (wkv[:, 0], wkv[:, 1], wkv[:, 2],
                                wkv[:, 3])
            wo8 = persist.tile([128, 2, 2, E], FP8, tag="wo8")
            dwo = persist.tile([128, 2, 2, E], FP8, tag="dwo")
            qtb = [persist.tile([128, S], BF16, tag=f"qtb{j}", name=f"qtb{j}")
                   for j in range(4)]
            qt8 = [persist.tile([128, S], FP8, tag=f"qt8{j}", name=f"qt8{j}")
                   for j in range(4)]
            ktb = persist.tile([128, S], BF16, tag="ktb")
            kt8 = persist.tile([128, 2, S], FP8, tag="kt8")
            vtb = persist.tile([128, 2, NKB, D + 1], BF16, tag="vtb")
            vt8 = persist.tile([128, 2, NKB, D + 1], FP8, tag="vt8")
            attnT8 = [persist.tile([128, 2, S], FP8, tag=f"attnT8{j}",
                                   name=f"attnT8{j}") for j in range(2)]
            datT8 = [persist.tile([128, 2, S], FP8, tag=f"datT8{j}",
                                  name=f"datT8{j}") for j in range(2)]
            msk = persist.tile([128, 896], FP8, tag="msk")
            cmt = msk[:, 0:256].rearrange("p (a b) -> p a b", a=2)
            cmx = msk[:, 256:512].rearrange("p (a b) -> p a b", a=2)
            seli = msk[:, 512:640]
            selw = msk[:, 640:896]
            id128 = persist.tile([128, 128], BF16, tag="id128")
            id64 = persist.tile([128, D], BF16, tag="id64")
            bias_t = persist.tile([128, 8], F32, tag="bias")

            # ---- input DMAs, ordered so first streams start early ----
            nc.sync.dma_start(out=wq[:, 0], in_=WQ[:, 0])
            nc.sync.dma_start(out=xt[:, :, :, 0:512], in_=XT[:, :, :, 0:512])
            nc.sync.dma_start(out=wkv[:, 0:2], in_=WKV[:, 0:2])
            nc.sync.dma_start(out=xr[:, :, :, 0:512], in_=XR[:, :, :, 0:512])
            nc.sync.dma_start(out=wqr[:, 0], in_=WQR[:, 0])
            nc.sync.dma_start(out=bias_t, in_=BIAS[:, :])
            nc.sync.dma_start(out=msk, in_=MSK[:, :])
            nc.sync.dma_start(out=wkv[:, 2:4], in_=WKV[:, 2:4])
            nc.sync.dma_start(out=wq[:, 1], in_=WQ[:, 1])
            nc.sync.dma_start(out=wqr[:, 1], in_=WQR[:, 1])
            nc.sync.dma_start(out=xt[:, :, :, 512:1024],
                              in_=XT[:, :, :, 512:1024])
            nc.sync.dma_start(out=xr[:, :, :, 512:1024],
                              in_=XR[:, :, :, 512:1024])
            nc.sync.dma_start(out=wq[:, 2], in_=WQ[:, 2])
            nc.sync.dma_start(out=wq[:, 3], in_=WQ[:, 3])
            nc.sync.dma_start(out=wqr[:, 2], in_=WQR[:, 2])
            nc.sync.dma_start(out=wqr[:, 3], in_=WQR[:, 3])
            nc.sync.dma_start(out=xt[:, :, :, 1024:1536],
                              in_=XT[:, :, :, 1024:1536])
            nc.sync.dma_start(out=xt[:, :, :, 1536:2048],
                              in_=XT[:, :, :, 1536:2048])
            nc.sync.dma_start(out=xr[:, :, :, 1024:1536],
                              in_=XR[:, :, :, 1024:1536])
            nc.sync.dma_start(out=xr[:, :, :, 1536:2048],
                              in_=XR[:, :, :, 1536:2048])
            nc.sync.dma_start(out=wo8, in_=WO8[:, :])
            nc.sync.dma_start(out=dwo, in_=DWO[:, :])

            make_identity(nc, id128)
            make_identity(nc, id64[0:64, :])
            make_identity(nc, id64[64:128, :])
            nc.gpsimd.memset(kt8[:, 1, :], 0.0)        # zero K plane
            nc.gpsimd.memset(vtb[:, :, :, D:D + 1], 1.0)   # denom ones
            nc.gpsimd.memset(vt8[:, :, :, D:D + 1], 1.0)

            # ---------------- comp3 projection fills ----------------
            def comp3(st, w8, wr, cols, n):
                """3-term compensated projection as 3 PE sub-units."""
                def u1():
                    st["ps1"] = wkp.tile([128, n], F32, tag="wk", name="ps1")
                    for c2 in range(8):
                        nc.tensor.matmul(
                            st["ps1"], w8[:, c2], xt[:, c2, :, cols],
                            start=(c2 == 0), stop=(c2 == 7),
                            perf_mode=DR, skip_group_check=True)

                def u2():
                    st["ps2"] = wkp.tile([128, n], F32, tag="wk", name="ps2")
                    for c2 in range(8):
                        nc.tensor.matmul(
                            st["ps2"], w8[:, c2], xr[:, c2, :, cols],
                            start=(c2 == 0), stop=False,
                            perf_mode=DR, skip_group_check=True)

                def u3():
                    for c2 in range(8):
                        nc.tensor.matmul(
                            st["ps2"], wr[:, c2], xt[:, c2, :, cols],
                            start=False, stop=(c2 == 7),
                            perf_mode=DR, skip_group_check=True)
                return [u1, u2, u3]

            def merge(st, out_b, bcol, n):
                """DVE: out_b = ps1 + ps2/32 + bias."""
                tm = tmpp.tile([128, 512], F32, tag="tm", name="tm")
                nc.vector.tensor_scalar(
                    tm[:, 0:n], st["ps2"], 1.0 / 32.0,
                    bias_t[:, bcol:bcol + 1], MULT, ADD)
                nc.vector.tensor_add(out_b, st["ps1"], tm[:, 0:n])

            def make_qproj(t5, jb):
                cols = bass.ds(t5 * 512, 512)
                st = {}

                def seg(which, c0, alloc=False, start=False, stop=False):
                    def f():
                        if alloc:
                            st[which] = wkp.tile([128, 512], F32, tag="wk",
                                                 name=which)
                        w = wq[:, jb] if which == "ps1" or c0 >= 100                             else wq[:, jb]
                        for c2 in range(c0 % 100, c0 % 100 + 4):
                            if which == "ps1":
                                lhs, rhs = wq[:, jb, c2], xt[:, c2, :, cols]
                            elif c0 < 100:
                                lhs, rhs = wq[:, jb, c2], xr[:, c2, :, cols]
                            else:
                                lhs, rhs = wqr[:, jb, c2], xt[:, c2, :, cols]
                            nc.tensor.matmul(
                                st[which], lhs, rhs,
                                start=(start and c2 == c0 % 100),
                                stop=(stop and c2 == c0 % 100 + 3),
                                perf_mode=DR, skip_group_check=True)
                    return f

                def fin():
                    seg("ps2", 104, stop=True)()
                    merge(st, qtb[jb][:, cols], jb, 512)
                    nc.gpsimd.tensor_copy(qt8[jb][:, cols], qtb[jb][:, cols])
                return [seg("ps1", 0, alloc=True, start=True),
                        seg("ps1", 4, stop=True),
                        seg("ps2", 0, alloc=True, start=True),
                        seg("ps2", 4),
                        seg("ps2", 100),
                        fin]

            def make_kproj(tg):
                cols = bass.ds(tg * 256, 256)
                st = {}
                us = comp3(st, wk, wkr, cols, 256)

                def fin():
                    us[2]()
                    merge(st, ktb[:, cols], 4, 256)
                    nc.gpsimd.tensor_copy(kt8[:, 0, cols], ktb[:, cols])
                return [us[0], us[1], fin]

            def make_vproj(tg):
                cols = bass.ds(tg * 256, 256)
                st = {}
                us = comp3(st, wv, wvr, cols, 256)

                def fin():
                    us[2]()
                    st["vs"] = vsp.tile([128, 256], BF16, tag="vs",
                                        name="vs")
                    merge(st, st["vs"], 5, 256)

                def transp(kv):
                    def f():
                        tp = wkp.tile([128, 2, D], BF16, tag="wk",
                                      name="vtp")
                        for tc2 in range(2):
                            nc.tensor.transpose(
                                tp[:, tc2, :],
                                st["vs"][kv * 64:kv * 64 + 64,
                                         tc2 * 128:(tc2 + 1) * 128],
                                id64[kv * 64:kv * 64 + 64, :])
                        nc.vector.tensor_copy(
                            vtb[:, kv, 2 * tg:2 * tg + 2, 0:D], tp)
                        nc.gpsimd.tensor_copy(
                            vt8[:, kv, 2 * tg:2 * tg + 2, 0:D],
                            vtb[:, kv, 2 * tg:2 * tg + 2, 0:D])
                    return f
                return [us[0], us[1], fin, transp(0), transp(1)]

            osp_tiles = {}
            Copy = mybir.ActivationFunctionType.Copy

            def emit_oproj_chunk(tb, ng, on_act=False):
                if tb not in osp_tiles:
                    osp_tiles[tb] = osp.tile([128, E], BF16, tag="os",
                                             name="ostage")
                op = wkp.tile([128, 512], F32, tag="wk", name="opc")
                tc_ = bass.ds(tb * 128, 128)
                ngc = bass.ds(ng * 512, 512)
                for ti, (lh, rh) in enumerate(
                        [(attnT8, wo8), (datT8, wo8), (attnT8, dwo)]):
                    for jbp in range(2):
                        nc.tensor.matmul(
                            op, lh[jbp][:, :, tc_], rh[:, jbp, :, ngc],
                            start=(ti == 0 and jbp == 0),
                            stop=(ti == 2 and jbp == 1),
                            perf_mode=DR, skip_group_check=True)
                if on_act:
                    nc.scalar.activation(
                        osp_tiles[tb][:, ng * 512:(ng + 1) * 512], op,
                        Copy, bias=0.0, scale=bias_t[:, 6:7])
                else:
                    nc.vector.tensor_scalar_mul(
                        osp_tiles[tb][:, ng * 512:(ng + 1) * 512], op,
                        bias_t[:, 6:7])
                if ng == 3:
                    nc.sync.dma_start(out=OUT[tb * 128:(tb + 1) * 128, :],
                                      in_=osp_tiles[tb])

            def qu(t5, jb):
                return make_qproj(t5, jb)

            def kvu(tg):
                return make_kproj(tg) + make_vproj(tg)

            op_st = {}

            def emit_oproj_half(tb, ng, half):
                key = (tb, ng)
                if half == 0:
                    if tb not in osp_tiles:
                        osp_tiles[tb] = osp.tile([128, E], BF16, tag="os",
                                                 name="ostage")
                    op_st[key] = wkp.tile([128, 512], F32, tag="wk",
                                          name="opc")
                op = op_st[key]
                tc_ = bass.ds(tb * 128, 128)
                ngc = bass.ds(ng * 512, 512)
                terms = [(attnT8, wo8, 0), (attnT8, wo8, 1),
                         (datT8, wo8, 0), (datT8, wo8, 1),
                         (attnT8, dwo, 0), (attnT8, dwo, 1)]
                rng = terms[0:3] if half == 0 else terms[3:6]
                for ti, (lh, rh, jbp) in enumerate(rng):
                    nc.tensor.matmul(
                        op, lh[jbp][:, :, tc_], rh[:, jbp, :, ngc],
                        start=(half == 0 and ti == 0),
                        stop=(half == 1 and ti == 2),
                        perf_mode=DR, skip_group_check=True)
                if half == 1:
                    nc.vector.tensor_scalar_mul(
                        osp_tiles[tb][:, ng * 512:(ng + 1) * 512], op,
                        bias_t[:, 6:7])
                    if ng == 3:
                        nc.sync.dma_start(
                            out=OUT[tb * 128:(tb + 1) * 128, :],
                            in_=osp_tiles[tb])
                    del op_st[key]

            def ou(tbs):
                return [(lambda tb=tb, ng=ng, hf=hf:
                         emit_oproj_half(tb, ng, hf))
                        for tb in tbs for ng in range(4) for hf in (0, 1)]

            # ---------------- attention streams ----------------
            def emit_stream_scores(at_t, qg, h, kv):
                nkb = 4 * qg + 4
                npair = nkb // 2
                qoff = kv * 64
                q0 = qg * 512

                def score8(bank_ap, kb, c0, start, stop):
                    n = 512 - c0
                    nc.tensor.matmul(
                        bank_ap[:, c0:512],
                        kt8[qoff:qoff + 64, :, kb * 128:(kb + 1) * 128],
                        qt8[h][qoff:qoff + 64, q0 + c0:q0 + 512]
                        .unsqueeze(1).broadcast_to([64, 2, n]),
                        start=start, stop=stop,
                        perf_mode=DR, skip_group_check=True)

                def scoreb(bank_ap, kb, c0, start, stop):
                    nc.tensor.matmul(
                        bank_ap[:, c0:512],
                        ktb[qoff:qoff + 64, kb * 128:(kb + 1) * 128],
                        qtb[h][qoff:qoff + 64, q0 + c0:q0 + 512],
                        start=start, stop=stop, skip_group_check=True)

                score = scoreb if qg == 0 else score8

                def mask_tril(bank_ap, c0):
                    nc.tensor.matmul(
                        bank_ap[:, c0:c0 + 128], cmt,
                        seli.unsqueeze(1).broadcast_to([128, 2, 128]),
                        start=True, stop=False,
                        perf_mode=DR, skip_group_check=True)

                for pr in range(npair):
                    kb0 = 2 * pr
                    di = pr - (npair - 2)  # 0 => {j0,j1}, 1 => {j2,j3}
                    sc = scp.tile([128, 2, 512], F32, tag="sc", name="sc")
                    if di < 0:
                        for b in range(2):
                            score(sc[:, b], kb0 + b, 0, True, True)
                    elif di == 0:
                        mask_tril(sc[:, 0], 0)
                        score(sc[:, 0], kb0, 0, False, True)
                        nc.tensor.matmul(
                            sc[:, 1, 0:256], cmx,
                            selw.unsqueeze(1).broadcast_to([128, 2, 256]),
                            start=True, stop=False,
                            perf_mode=DR, skip_group_check=True)
                        score(sc[:, 1], kb0 + 1, 128, False, True)
                    else:
                        mask_tril(sc[:, 0], 256)
                        score(sc[:, 0], kb0, 256, False, True)
                        # j3: cols [256:384) fully masked + tril [384:512)
                        nc.tensor.matmul(
                            sc[:, 1, 256:512], cmx,
                            selw.unsqueeze(1).broadcast_to([128, 2, 256]),
                            start=True, stop=False,
                            perf_mode=DR, skip_group_check=True)
                        score(sc[:, 1], kb0 + 1, 384, False, True)
                    if di == 1:
                        nc.scalar.activation(
                            at_t[:, kb0:kb0 + 2, 256:512],
                            sc[:, :, 256:512], Exp,
                            bias=bias_t[:, 7:8], scale=1.0 / 8192.0)
                    else:
                        nc.scalar.activation(
                            at_t[:, kb0:kb0 + 2, :], sc, Exp,
                            bias=bias_t[:, 7:8], scale=1.0 / 8192.0)
                    yield

            def emit_avburst(at_t, qg, h, kv, an_t):
                nkb = 4 * qg + 4
                npair = nkb // 2
                for s in range(4):
                    slot = wkp.tile([128, D + 1], F32, tag="wk", name="av")
                    if qg == 0:
                        for kb in range(nkb):
                            nc.tensor.matmul(
                                slot,
                                at_t[:, kb, s * 128:(s + 1) * 128],
                                vtb[:, kv, kb, 0:D + 1],
                                start=(kb == 0), stop=(kb == nkb - 1),
                                skip_group_check=True)
                    else:
                        for pr in range(npair):
                            nc.tensor.matmul(
                                slot,
                                at_t[:, 2 * pr:2 * pr + 2,
                                     s * 128:(s + 1) * 128],
                                vt8[:, kv, 2 * pr:2 * pr + 2, 0:D + 1],
                                start=(pr == 0), stop=(pr == npair - 1),
                                perf_mode=DR, skip_group_check=True)
                    rl = rlp.tile([128, 1], F32, tag="rl", name="rl")
                    nc.vector.reciprocal_approx_fast(rl, slot[:, D:D + 1])
                    nc.vector.tensor_scalar_mul(
                        an_t[:, s, kv * 64:kv * 64 + 64], slot[:, 0:D],
                        rl[:, 0:1])

            def emit_antranspose(an_t, qg, h):
                q0 = qg * 512
                jbp, pl = h // 2, h % 2
                for qb in range(4):
                    tp = wkp.tile([128, 128], BF16, tag="wk", name="atp2")
                    nc.tensor.transpose(tp, an_t[:, qb, :], id128)
                    cols = bass.ds(q0 + qb * 128, 128)
                    ab = antp.tile([128, 128], BF16, tag="ab", name="ab")
                    nc.vector.tensor_copy(ab, tp)
                    nc.gpsimd.tensor_copy(attnT8[jbp][:, pl, cols], ab)
                    nc.gpsimd.tensor_sub(datT8[jbp][:, pl, cols], ab,
                                         attnT8[jbp][:, pl, cols])

            # ---------------- phase 1: first-stream prereqs ----------------
            for f in qu(0, 0) + kvu(0) + kvu(1):
                f()

            fills = {
                0: qu(0, 1) + qu(0, 2) + qu(0, 3) + qu(1, 0) + qu(1, 1) +
                   kvu(2) + kvu(3),
                1: qu(1, 2) + qu(1, 3) + qu(2, 0) + qu(2, 1) +
                   kvu(4) + kvu(5),
                2: qu(2, 2) + qu(2, 3) + qu(3, 0) + qu(3, 1) +
                   kvu(6) + kvu(7) + ou((0, 1)),
                3: qu(3, 2) + qu(3, 3) + ou((2, 3)) + ou((4, 5)) +
                   ou((6, 7)) + ou((8, 9, 10, 11)),
            }

            # ---------------- main stream loop ----------------
            prev = None          # (at_t, qg, h, kv, an_t)
            an_cur = {}
            pend_tr = []

            for qg in range(4):
                units = list(fills[qg])
                nu = len(units)
                done = 0
                npair = 2 * qg + 2
                nstep = 8 * npair
                for si, (h, kv) in enumerate([(h, kv) for h in range(4)
                                              for kv in range(2)]):
                    nkb = 4 * qg + 4
                    if qg == 0:
                        at_t = atb.tile([128, 4, 512], BF16, tag="atb",
                                        name="atb")
                    else:
                        at_t = atf.tile([128, NKB, 512], FP8, tag="atf",
                                        name="atf")
                    # zero never-exp'd rects of the {j2,j3} diag pair
                    nc.gpsimd.memset(at_t[:, nkb - 2:nkb, 0:256], 0.0)
                    if kv == 0:
                        an_cur[h] = anp.tile([128, 4, 128], BF16, tag="an",
                                             name="an2")
                    gen = emit_stream_scores(at_t, qg, h, kv)
                    step = 0
                    for _ in gen:
                        step += 1
                        # lagged work tucked behind this stream's first exps
                        if step == 1 and prev is not None:
                            emit_avburst(*prev)
                            if prev[3] == 1:
                                pend_tr.append((prev[4], prev[1], prev[2]))
                            prev = None
                        if step == 2:
                            while len(pend_tr) > 1:
                                a, g, hh = pend_tr.pop(0)
                                emit_antranspose(a, g, hh)
                        want = nu * (si * npair + step) // nstep
                        while done < want:
                            units[done]()
                            done += 1
                    prev = (at_t, qg, h, kv, an_cur[h])
                while done < nu:
                    units[done]()
                    done += 1

            # ---------------- tail ----------------
            emit_avburst(*prev)
            pend_tr.append((prev[4], prev[1], prev[2]))
            while pend_tr:
                a, g, hh = pend_tr.pop(0)
                emit_antranspose(a, g, hh)
            for tb in (12, 13, 14, 15):
                for ng in range(4):
                    emit_oproj_chunk(tb, ng, on_act=(ng % 2 == 1))

    nc.compile()
    return nc


def _prep_core_inputs(c, x, Wq, bq, Wk, bk, Wv, bv, Wo, xt_cache, fp8):
    import ml_dtypes
    bf16 = ml_dtypes.bfloat16
    g = c % 4
    b = c // 4
    f32 = np.float32
    if b not in xt_cache:
        # xt[p, c2, i, t] = x[b, t, 128*(2*c2+i)+p]; xr = 32*residual
        xm = np.ascontiguousarray(
            x[b].T.reshape(16, 128, S).reshape(8, 2, 128, S)
            .transpose(2, 0, 1, 3))
        x8 = xm.astype(fp8)
        xrr = ((xm - x8.astype(f32)) * 32.0).astype(fp8)
        xt_cache[b] = (x8, xrr)
    x8, xrr = xt_cache[b]

    def split8(wm):
        w8 = wm.astype(fp8)
        wr = ((wm - w8.astype(f32)) * 32.0).astype(fp8)
        return w8, wr

    wq_s = Wq[:, 512 * g:512 * (g + 1)].reshape(E, 8, 64)
    wq_s = wq_s[:, HEAD_PERM, :].reshape(E, 512) * f32(WS)
    wq_m = np.ascontiguousarray(
        wq_s.reshape(8, 2, 128, 4, 128).transpose(2, 3, 0, 1, 4))
    wq8, wqr = split8(wq_m)
    wk_s = Wk[:, 128 * g:128 * (g + 1)] * f32(WS)
    wk_m = np.ascontiguousarray(
        wk_s.reshape(8, 2, 128, 128).transpose(2, 0, 1, 3))
    wk8, wkr = split8(wk_m)
    wv_s = Wv[:, 128 * g:128 * (g + 1)] * f32(WS)
    wv_m = np.ascontiguousarray(
        wv_s.reshape(8, 2, 128, 128).transpose(2, 0, 1, 3))
    wv8, wvr = split8(wv_m)
    wo_s = Wo[512 * g:512 * (g + 1), :].reshape(8, 64, E)
    wo_s = wo_s[HEAD_PERM, :, :].reshape(512, E) * f32(WS)
    wo_m = np.ascontiguousarray(
        wo_s.reshape(2, 2, 128, E).transpose(2, 0, 1, 3))
    wo8 = wo_m.astype(fp8)
    dwo = (wo_m - wo8.astype(f32)).astype(fp8)
    bias = np.zeros((128, 8), f32)
    bq_s = bq[512 * g:512 * (g + 1)].reshape(8, 64)[HEAD_PERM, :].reshape(512)
    bias[:, 0:4] = bq_s.reshape(4, 128).T * WS
    bias[:, 4] = bk[128 * g:128 * (g + 1)] * WS
    bias[:, 5] = bv[128 * g:128 * (g + 1)] * WS
    bias[:, 6] = 1.0 / 1024.0
    bias[:, 7] = -3.3
    pp = np.arange(128)[:, None]
    kk = np.arange(128)[None, :]
    cmt = np.repeat(np.where(pp < kk, -240.0, 0.0)[:, None, :],
                    2, axis=1).astype(fp8)
    cmx = np.repeat(np.where(pp <= kk, -240.0, 0.0)[:, None, :],
                    2, axis=1).astype(fp8)
    seli = (np.eye(128, dtype=f32) * 240.0).astype(fp8)
    selw = np.zeros((128, 256), f32)
    selw[0, 0:128] = 240.0
    for cc in range(128, 255):
        selw[cc - 127, cc] = 240.0
    selw = selw.astype(fp8)
    wkv = np.stack([wk8, wkr, wv8, wvr], axis=1)
    msk = np.concatenate(
        [cmt.reshape(128, 256), cmx.reshape(128, 256), seli, selw],
        axis=1)
    return {"xt": x8, "xr": xrr, "wq": wq8, "wqr": wqr, "wkv": wkv,
            "wo8": wo8, "dwo": dwo, "bias": bias, "msk": msk}


def kernel(**inputs):
    import ml_dtypes
    from concourse.bass_utils import run_bass_kernel_spmd

    fp8 = ml_dtypes.float8_e4m3
    x = np.asarray(inputs["x"], np.float32)
    Wq = np.asarray(inputs["Wq"], np.float32)
    bq = np.asarray(inputs["bq"], np.float32)
    Wk = np.asarray(inputs["Wk"], np.float32)
    bk = np.asarray(inputs["bk"], np.float32)
    Wv = np.asarray(inputs["Wv"], np.float32)
    bv = np.asarray(inputs["bv"], np.float32)
    Wo = np.asarray(inputs["Wo"], np.float32)
    bo = np.asarray(inputs["bo"], np.float32)

    if "nc" not in _CACHE:
        _CACHE["nc"] = _build()
    nc = _CACHE["nc"]

    xt_cache = {}
    in_maps = [_prep_core_inputs(c, x, Wq, bq, Wk, bk, Wv, bv, Wo,
                                 xt_cache, fp8)
               for c in range(NCORE)]
    res = run_bass_kernel_spmd(nc, in_maps, list(range(NCORE)))
    parts = [res.results[c]["out"].astype(np.float32) for c in range(NCORE)]
    out0 = parts[0] + parts[1] + parts[2] + parts[3] + bo
    out1 = parts[4] + parts[5] + parts[6] + parts[7] + bo
    return np.stack([out0, out1]).astype(np.float32)


# revision 29
# speedup vs baseline: 1.0048x; 1.0013x over previous
"""GQA kernel for Trainium2, 8 NeuronCores — mixed bf16/fp8-DoubleRow.

Sharding: core c -> batch b = c//4, kv-head-group g = c%4.
Each core: 1 batch, 2 KV heads (2g, 2g+1), 8 Q heads, row-shard of W_o
(rows 512g..512g+512). Host sums the 4 partials per batch, /1024, + bo.

Precision plan (gate: rel err < 2e-2 vs abs-max):
  - Q/K/V projections: 3-term error-compensated fp8 DoubleRow,
      P = x8@W8  (psum1)   +   [(32dx)8@W8 + x8@(32dW)8]/32  (psum2),
    residuals prepped on host, merged by DVE into bf16 (error ~0.1%).
    Weights carry a x32 base scale for fp8 range; Q/K/V come out x32.
  - Scores + A@V for q-tiles 1..3 (n_eff > 512): fp8 DoubleRow from
    fp8 copies of Q/K/at/V — softmax normalization suppresses these
    errors by ~1/sqrt(n_eff).
  - Scores + A@V for q-tile 0 and the O projection: bf16 (quantization
    there is not normalization-suppressed).
  - exp bias -3.3 keeps fp8 'at' in [3e-8, 81] (e4m3 max 240).
  - Causal masks: PE matmuls with +-240 fp8 constants (2-plane product
    -115200 -> exp ~ 0).

Streams (h, kv, qg): key-block PAIRS fill a [128,2,512] psum group
(2-bank, ping-pong) -> one wide exp on ACT into at2 ring buffers.
Diagonal quads split {j0,j1} (full width) / {j2,j3} (cols 256:512,
rects memset 0).  A@V runs lagged one stream from saved at2: 4 q-subtile
slots sequentially through the work-psum ring, DVE recip+mul -> an2
(bf16), PE transpose -> attnT for the bf16 O projection.  O-proj psum
chunks DMA straight to DRAM as f32 (host unscales 1/1024).
Projection/O chunks stream between attention pairs as budgeted fills.

PSUM: 4 banks score ping-pong + 4-bank work ring.
"""

import numpy as np

E = 2048
S = 2048
B = 2
D = 64
NCORE = 8
NKB = S // 128      # 16 key blocks
WS = 32.0           # host base scale for all weights

_CACHE = {}
# tile jb holds q-heads (jb, jb+4): kv0 head dims at partitions 0:64,
# kv1 head dims at 64:128, matching the K/V partition layout
HEAD_PERM = [0, 4, 1, 5, 2, 6, 3, 7]


def _build():
    import concourse.bass as bass
    import concourse.tile as tile
    from concourse import mybir
    from concourse import bacc
    from concourse.masks import make_identity

    F32 = mybir.dt.float32
    BF16 = mybir.dt.bfloat16
    FP8 = mybir.dt.float8e4
    Exp = mybir.ActivationFunctionType.Exp
    DR = mybir.MatmulPerfMode.DoubleRow
    ADD = mybir.AluOpType.add
    MULT = mybir.AluOpType.mult

    nc = bacc.Bacc("TRN2", target_bir_lowering=False, debug=False,
                   num_devices=NCORE)

    XT = nc.declare_dram_parameter("xt", [128, 8, 2, S], FP8, isOutput=False)
    XR = nc.declare_dram_parameter("xr", [128, 8, 2, S], FP8, isOutput=False)
    WQ = nc.declare_dram_parameter("wq", [128, 4, 8, 2, 128], FP8,
                                   isOutput=False)
    WQR = nc.declare_dram_parameter("wqr", [128, 4, 8, 2, 128], FP8,
                                    isOutput=False)
    WKV = nc.declare_dram_parameter("wkv", [128, 4, 8, 2, 128], FP8,
                                    isOutput=False)
    WO8 = nc.declare_dram_parameter("wo8", [128, 2, 2, E], FP8,
                                    isOutput=False)
    DWO = nc.declare_dram_parameter("dwo", [128, 2, 2, E], FP8,
                                    isOutput=False)
    BIAS = nc.declare_dram_parameter("bias", [128, 8], F32, isOutput=False)
    MSK = nc.declare_dram_parameter("msk", [128, 896], FP8, isOutput=False)
    OUT = nc.declare_dram_parameter("out", [S, E], BF16, isOutput=True)

    with tile.TileContext(nc) as tc:
        with tc.tile_pool(name="persist", bufs=1) as persist, \
             tc.tile_pool(name="atf", bufs=2) as atf, \
             tc.tile_pool(name="atb", bufs=2) as atb, \
             tc.tile_pool(name="anp", bufs=2) as anp, \
             tc.tile_pool(name="vsp", bufs=2) as vsp, \
             tc.tile_pool(name="rlp", bufs=4) as rlp, \
             tc.tile_pool(name="tmp", bufs=2) as tmpp, \
             tc.tile_pool(name="osp", bufs=2) as osp, \
             tc.tile_pool(name="antp", bufs=4) as antp, \
             tc.tile_pool(name="scp", bufs=2, space="PSUM") as scp, \
             tc.tile_pool(name="wkp", bufs=4, space="PSUM") as wkp:

            # ---- persistent SBUF ----
            xt = persist.tile([128, 8, 2, S], FP8, tag="xt")
            xr = persist.tile([128, 8, 2, S], FP8, tag="xr")
            wq = persist.tile([128, 4, 8, 2, 128], FP8, tag="wq")
            wqr = persist.tile([128, 4, 8, 2, 128], FP8, tag="wqr")
            wkv = persist.tile([128, 4, 8, 2, 128], FP8, tag="wkv")
            wk, wkr, wv, wvr = (wkv[:, 0], wkv[:, 1], wkv[:, 2],
                                wkv[:, 3])
            wo8 = persist.tile([128, 2, 2, E], FP8, tag="wo8")
            dwo = persist.tile([128, 2, 2, E], FP8, tag="dwo")
            qtb = [persist.tile([128, S], BF16, tag=f"qtb{j}", name=f"qtb{j}")
                   for j in range(4)]
            qt8 = [persist.tile([128, S], FP8, tag=f"qt8{j}", name=f"qt8{j}")
                   for j in range(4)]
            ktb = persist.tile([128, S], BF16, tag="ktb")
            kt8 = persist.tile([128, 2, S], FP8, tag="kt8")
            vtb = persist.tile([128, 2, NKB, D + 1], BF16, tag="vtb")
            vt8 = persist.tile([128, 2, NKB, D + 1], FP8, tag="vt8")
            attnT8 = [persist.tile([128, 2, S], FP8, tag=f"attnT8{j}",
                                   name=f"attnT8{j}") for j in range(2)]
            datT8 = [persist.tile([128, 2, S], FP8, tag=f"datT8{j}",
                                  name=f"datT8{j}") for j in range(2)]
            msk = persist.tile([128, 896], FP8, tag="msk")
            cmt = msk[:, 0:256].rearrange("p (a b) -> p a b", a=2)
            cmx = msk[:, 256:512].rearrange("p (a b) -> p a b", a=2)
            seli = msk[:, 512:640]
            selw = msk[:, 640:896]
            id128 = persist.tile([128, 128], BF16, tag="id128")
            id64 = persist.tile([128, D], BF16, tag="id64")
            bias_t = persist.tile([128, 8], F32, tag="bias")

            # ---- input DMAs, ordered so first streams start early ----
            nc.sync.dma_start(out=wq[:, 0], in_=WQ[:, 0])
            nc.sync.dma_start(out=xt[:, :, :, 0:512], in_=XT[:, :, :, 0:512])
            nc.sync.dma_start(out=wkv[:, 0:2], in_=WKV[:, 0:2])
            nc.sync.dma_start(out=xr[:, :, :, 0:512], in_=XR[:, :, :, 0:512])
            nc.sync.dma_start(out=wqr[:, 0], in_=WQR[:, 0])
            nc.sync.dma_start(out=bias_t, in_=BIAS[:, :])
            nc.sync.dma_start(out=msk, in_=MSK[:, :])
            nc.sync.dma_start(out=wkv[:, 2:4], in_=WKV[:, 2:4])
            nc.sync.dma_start(out=wq[:, 1], in_=WQ[:, 1])
            nc.sync.dma_start(out=wqr[:, 1], in_=WQR[:, 1])
            nc.sync.dma_start(out=xt[:, :, :, 512:1024],
                              in_=XT[:, :, :, 512:1024])
            nc.sync.dma_start(out=xr[:, :, :, 512:1024],
                              in_=XR[:, :, :, 512:1024])
            nc.sync.dma_start(out=wq[:, 2], in_=WQ[:, 2])
            nc.sync.dma_start(out=wq[:, 3], in_=WQ[:, 3])
            nc.sync.dma_start(out=wqr[:, 2], in_=WQR[:, 2])
            nc.sync.dma_start(out=wqr[:, 3], in_=WQR[:, 3])
            nc.sync.dma_start(out=xt[:, :, :, 1024:1536],
                              in_=XT[:, :, :, 1024:1536])
            nc.sync.dma_start(out=xt[:, :, :, 1536:2048],
                              in_=XT[:, :, :, 1536:2048])
            nc.sync.dma_start(out=xr[:, :, :, 1024:1536],
                              in_=XR[:, :, :, 1024:1536])
            nc.sync.dma_start(out=xr[:, :, :, 1536:2048],
                              in_=XR[:, :, :, 1536:2048])
            nc.sync.dma_start(out=wo8, in_=WO8[:, :])
            nc.sync.dma_start(out=dwo, in_=DWO[:, :])

            make_identity(nc, id128)
            make_identity(nc, id64[0:64, :])
            make_identity(nc, id64[64:128, :])
            nc.gpsimd.memset(kt8[:, 1, :], 0.0)        # zero K plane
            nc.gpsimd.memset(vtb[:, :, :, D:D + 1], 1.0)   # denom ones
            nc.gpsimd.memset(vt8[:, :, :, D:D + 1], 1.0)

            # ---------------- comp3 projection fills ----------------
            def comp3(st, w8, wr, cols, n):
                """3-term compensated projection as 3 PE sub-units."""
                def u1():
                    st["ps1"] = wkp.tile([128, n], F32, tag="wk", name="ps1")
                    for c2 in range(8):
                        nc.tensor.matmul(
                            st["ps1"], w8[:, c2], xt[:, c2, :, cols],
                            start=(c2 == 0), stop=(c2 == 7),
                            perf_mode=DR, skip_group_check=True)

                def u2():
                    st["ps2"] = wkp.tile([128, n], F32, tag="wk", name="ps2")
                    for c2 in range(8):
                        nc.tensor.matmul(
                            st["ps2"], w8[:, c2], xr[:, c2, :, cols],
                            start=(c2 == 0), stop=False,
                            perf_mode=DR, skip_group_check=True)

                def u3():
                    for c2 in range(8):
                        nc.tensor.matmul(
                            st["ps2"], wr[:, c2], xt[:, c2, :, cols],
                            start=False, stop=(c2 == 7),
                            perf_mode=DR, skip_group_check=True)
                return [u1, u2, u3]

            def merge(st, out_b, bcol, n):
                """DVE: out_b = ps1 + ps2/32 + bias."""
                tm = tmpp.tile([128, 512], F32, tag="tm", name="tm")
                nc.vector.tensor_scalar(
                    tm[:, 0:n], st["ps2"], 1.0 / 32.0,
                    bias_t[:, bcol:bcol + 1], MULT, ADD)
                nc.vector.tensor_add(out_b, st["ps1"], tm[:, 0:n])

            def make_qproj(t5, jb):
                cols = bass.ds(t5 * 512, 512)
                st = {}

                def seg(which, c0, alloc=False, start=False, stop=False):
                    def f():
                        if alloc:
                            st[which] = wkp.tile([128, 512], F32, tag="wk",
                                                 name=which)
                        w = wq[:, jb] if which == "ps1" or c0 >= 100                             else wq[:, jb]
                        for c2 in range(c0 % 100, c0 % 100 + 4):
                            if which == "ps1":
                                lhs, rhs = wq[:, jb, c2], xt[:, c2, :, cols]
                            elif c0 < 100:
                                lhs, rhs = wq[:, jb, c2], xr[:, c2, :, cols]
                            else:
                                lhs, rhs = wqr[:, jb, c2], xt[:, c2, :, cols]
                            nc.tensor.matmul(
                                st[which], lhs, rhs,
                                start=(start and c2 == c0 % 100),
                                stop=(stop and c2 == c0 % 100 + 3),
                                perf_mode=DR, skip_group_check=True)
                    return f

                def fin():
                    seg("ps2", 104, stop=True)()
                    merge(st, qtb[jb][:, cols], jb, 512)
                    nc.gpsimd.tensor_copy(qt8[jb][:, cols], qtb[jb][:, cols])
                return [seg("ps1", 0, alloc=True, start=True),
                        seg("ps1", 4, stop=True),
                        seg("ps2", 0, alloc=True, start=True),
                        seg("ps2", 4),
                        seg("ps2", 100),
                        fin]

            def make_kproj(tg):
                cols = bass.ds(tg * 256, 256)
                st = {}
                us = comp3(st, wk, wkr, cols, 256)

                def fin():
                    us[2]()
                    merge(st, ktb[:, cols], 4, 256)
                    nc.gpsimd.tensor_copy(kt8[:, 0, cols], ktb[:, cols])
                return [us[0], us[1], fin]

            def make_vproj(tg):
                cols = bass.ds(tg * 256, 256)
                st = {}
                us = comp3(st, wv, wvr, cols, 256)

                def fin():
                    us[2]()
                    st["vs"] = vsp.tile([128, 256], BF16, tag="vs",
                                        name="vs")
                    merge(st, st["vs"], 5, 256)

                def transp(kv):
                    def f():
                        tp = wkp.tile([128, 2, D], BF16, tag="wk",
                                      name="vtp")
                        for tc2 in range(2):
                            nc.tensor.transpose(
                                tp[:, tc2, :],
                                st["vs"][kv * 64:kv * 64 + 64,
                                         tc2 * 128:(tc2 + 1) * 128],
                                id64[kv * 64:kv * 64 + 64, :])
                        nc.vector.tensor_copy(
                            vtb[:, kv, 2 * tg:2 * tg + 2, 0:D], tp)
                        nc.gpsimd.tensor_copy(
                            vt8[:, kv, 2 * tg:2 * tg + 2, 0:D],
                            vtb[:, kv, 2 * tg:2 * tg + 2, 0:D])
                    return f
                return [us[0], us[1], fin, transp(0), transp(1)]

            osp_tiles = {}
            Copy = mybir.ActivationFunctionType.Copy

            def emit_oproj_chunk(tb, ng, on_act=False):
                if tb not in osp_tiles:
                    osp_tiles[tb] = osp.tile([128, E], BF16, tag="os",
                                             name="ostage")
                op = wkp.tile([128, 512], F32, tag="wk", name="opc")
                tc_ = bass.ds(tb * 128, 128)
                ngc = bass.ds(ng * 512, 512)
                for ti, (lh, rh) in enumerate(
                        [(attnT8, wo8), (datT8, wo8), (attnT8, dwo)]):
                    for jbp in range(2):
                        nc.tensor.matmul(
                            op, lh[jbp][:, :, tc_], rh[:, jbp, :, ngc],
                            start=(ti == 0 and jbp == 0),
                            stop=(ti == 2 and jbp == 1),
                            perf_mode=DR, skip_group_check=True)
                if on_act:
                    nc.scalar.activation(
                        osp_tiles[tb][:, ng * 512:(ng + 1) * 512], op,
                        Copy, bias=0.0, scale=bias_t[:, 6:7])
                else:
                    nc.vector.tensor_scalar_mul(
                        osp_tiles[tb][:, ng * 512:(ng + 1) * 512], op,
                        bias_t[:, 6:7])
                nc.sync.dma_start(
                    out=OUT[tb * 128:(tb + 1) * 128,
                            ng * 512:(ng + 1) * 512],
                    in_=osp_tiles[tb][:, ng * 512:(ng + 1) * 512])

            def qu(t5, jb):
                return make_qproj(t5, jb)

            def kvu(tg):
                return make_kproj(tg) + make_vproj(tg)

            op_st = {}

            def emit_oproj_half(tb, ng, half):
                key = (tb, ng)
                if half == 0:
                    if tb not in osp_tiles:
                        osp_tiles[tb] = osp.tile([128, E], BF16, tag="os",
                                                 name="ostage")
                    op_st[key] = wkp.tile([128, 512], F32, tag="wk",
                                          name="opc")
                op = op_st[key]
                tc_ = bass.ds(tb * 128, 128)
                ngc = bass.ds(ng * 512, 512)
                terms = [(attnT8, wo8, 0), (attnT8, wo8, 1),
                         (datT8, wo8, 0), (datT8, wo8, 1),
                         (attnT8, dwo, 0), (attnT8, dwo, 1)]
                rng = terms[0:3] if half == 0 else terms[3:6]
                for ti, (lh, rh, jbp) in enumerate(rng):
                    nc.tensor.matmul(
                        op, lh[jbp][:, :, tc_], rh[:, jbp, :, ngc],
                        start=(half == 0 and ti == 0),
                        stop=(half == 1 and ti == 2),
                        perf_mode=DR, skip_group_check=True)
                if half == 1:
                    nc.vector.tensor_scalar_mul(
                        osp_tiles[tb][:, ng * 512:(ng + 1) * 512], op,
                        bias_t[:, 6:7])
                    nc.sync.dma_start(
                        out=OUT[tb * 128:(tb + 1) * 128,
                                ng * 512:(ng + 1) * 512],
                        in_=osp_tiles[tb][:, ng * 512:(ng + 1) * 512])
                    del op_st[key]

            def ou(tbs):
                return [(lambda tb=tb, ng=ng, hf=hf:
                         emit_oproj_half(tb, ng, hf))
                        for tb in tbs for ng in range(4) for hf in (0, 1)]

            # ---------------- attention streams ----------------
            def emit_stream_scores(at_t, qg, h, kv):
                nkb = 4 * qg + 4
                npair = nkb // 2
                qoff = kv * 64
                q0 = qg * 512

                def score8(bank_ap, kb, c0, start, stop):
                    n = 512 - c0
                    nc.tensor.matmul(
                        bank_ap[:, c0:512],
                        kt8[qoff:qoff + 64, :, kb * 128:(kb + 1) * 128],
                        qt8[h][qoff:qoff + 64, q0 + c0:q0 + 512]
                        .unsqueeze(1).broadcast_to([64, 2, n]),
                        start=start, stop=stop,
                        perf_mode=DR, skip_group_check=True)

                def scoreb(bank_ap, kb, c0, start, stop):
                    nc.tensor.matmul(
                        bank_ap[:, c0:512],
                        ktb[qoff:qoff + 64, kb * 128:(kb + 1) * 128],
                        qtb[h][qoff:qoff + 64, q0 + c0:q0 + 512],
                        start=start, stop=stop, skip_group_check=True)

                score = scoreb if qg == 0 else score8

                def mask_tril(bank_ap, c0):
                    nc.tensor.matmul(
                        bank_ap[:, c0:c0 + 128], cmt,
                        seli.unsqueeze(1).broadcast_to([128, 2, 128]),
                        start=True, stop=False,
                        perf_mode=DR, skip_group_check=True)

                for pr in range(npair):
                    kb0 = 2 * pr
                    di = pr - (npair - 2)  # 0 => {j0,j1}, 1 => {j2,j3}
                    sc = scp.tile([128, 2, 512], F32, tag="sc", name="sc")
                    if di < 0:
                        for b in range(2):
                            score(sc[:, b], kb0 + b, 0, True, True)
                    elif di == 0:
                        mask_tril(sc[:, 0], 0)
                        score(sc[:, 0], kb0, 0, False, True)
                        nc.tensor.matmul(
                            sc[:, 1, 0:256], cmx,
                            selw.unsqueeze(1).broadcast_to([128, 2, 256]),
                            start=True, stop=False,
                            perf_mode=DR, skip_group_check=True)
                        score(sc[:, 1], kb0 + 1, 128, False, True)
                    else:
                        mask_tril(sc[:, 0], 256)
                        score(sc[:, 0], kb0, 256, False, True)
                        # j3: cols [256:384) fully masked + tril [384:512)
                        nc.tensor.matmul(
                            sc[:, 1, 256:512], cmx,
                            selw.unsqueeze(1).broadcast_to([128, 2, 256]),
                            start=True, stop=False,
                            perf_mode=DR, skip_group_check=True)
                        score(sc[:, 1], kb0 + 1, 384, False, True)
                    if di == 1:
                        nc.scalar.activation(
                            at_t[:, kb0:kb0 + 2, 256:512],
                            sc[:, :, 256:512], Exp,
                            bias=bias_t[:, 7:8], scale=1.0 / 8192.0)
                    else:
                        nc.scalar.activation(
                            at_t[:, kb0:kb0 + 2, :], sc, Exp,
                            bias=bias_t[:, 7:8], scale=1.0 / 8192.0)
                    yield

            def emit_avburst(at_t, qg, h, kv, an_t):
                nkb = 4 * qg + 4
                npair = nkb // 2
                for s in range(4):
                    slot = wkp.tile([128, D + 1], F32, tag="wk", name="av")
                    if qg == 0:
                        for kb in range(nkb):
                            nc.tensor.matmul(
                                slot,
                                at_t[:, kb, s * 128:(s + 1) * 128],
                                vtb[:, kv, kb, 0:D + 1],
                                start=(kb == 0), stop=(kb == nkb - 1),
                                skip_group_check=True)
                    else:
                        for pr in range(npair):
                            nc.tensor.matmul(
                                slot,
                                at_t[:, 2 * pr:2 * pr + 2,
                                     s * 128:(s + 1) * 128],
                                vt8[:, kv, 2 * pr:2 * pr + 2, 0:D + 1],
                                start=(pr == 0), stop=(pr == npair - 1),
                                perf_mode=DR, skip_group_check=True)
                    rl = rlp.tile([128, 1], F32, tag="rl", name="rl")
                    nc.vector.reciprocal_approx_fast(rl, slot[:, D:D + 1])
                    nc.vector.tensor_scalar_mul(
                        an_t[:, s, kv * 64:kv * 64 + 64], slot[:, 0:D],
                        rl[:, 0:1])

            def emit_antranspose(an_t, qg, h):
                q0 = qg * 512
                jbp, pl = h // 2, h % 2
                for qb in range(4):
                    tp = wkp.tile([128, 128], BF16, tag="wk", name="atp2")
                    nc.tensor.transpose(tp, an_t[:, qb, :], id128)
                    cols = bass.ds(q0 + qb * 128, 128)
                    ab = antp.tile([128, 128], BF16, tag="ab", name="ab")
                    nc.vector.tensor_copy(ab, tp)
                    nc.gpsimd.tensor_copy(attnT8[jbp][:, pl, cols], ab)
                    nc.gpsimd.tensor_sub(datT8[jbp][:, pl, cols], ab,
                                         attnT8[jbp][:, pl, cols])

            # ---------------- phase 1: first-stream prereqs ----------------
            for f in qu(0, 0) + kvu(0) + kvu(1):
                f()

            fills = {
                0: qu(0, 1) + qu(0, 2) + qu(0, 3) + qu(1, 0) + qu(1, 1) +
                   kvu(2) + kvu(3),
                1: qu(1, 2) + qu(1, 3) + qu(2, 0) + qu(2, 1) +
                   kvu(4) + kvu(5),
                2: qu(2, 2) + qu(2, 3) + qu(3, 0) + qu(3, 1) +
                   kvu(6) + kvu(7) + ou((0, 1)),
                3: qu(3, 2) + qu(3, 3) + ou((2, 3)) + ou((4, 5)) +
                   ou((6, 7)) + ou((8, 9, 10, 11)),
            }

            # ---------------- main stream loop ----------------
            prev = None          # (at_t, qg, h, kv, an_t)
            an_cur = {}
            pend_tr = []

            for qg in range(4):
                units = list(fills[qg])
                nu = len(units)
                done = 0
                npair = 2 * qg + 2
                nstep = 8 * npair
                for si, (h, kv) in enumerate([(h, kv) for h in range(4)
                                              for kv in range(2)]):
                    nkb = 4 * qg + 4
                    if qg == 0:
                        at_t = atb.tile([128, 4, 512], BF16, tag="atb",
                                        name="atb")
                    else:
                        at_t = atf.tile([128, NKB, 512], FP8, tag="atf",
                                        name="atf")
                    # zero never-exp'd rects of the {j2,j3} diag pair
                    nc.gpsimd.memset(at_t[:, nkb - 2:nkb, 0:256], 0.0)
                    if kv == 0:
                        an_cur[h] = anp.tile([128, 4, 128], BF16, tag="an",
                                             name="an2")
                    gen = emit_stream_scores(at_t, qg, h, kv)
                    step = 0
                    for _ in gen:
                        step += 1
                        # lagged work tucked behind this stream's first exps
                        if step == 1 and prev is not None:
                            emit_avburst(*prev)
                            if prev[3] == 1:
                                pend_tr.append((prev[4], prev[1], prev[2]))
                            prev = None
                        if step == 2:
                            while len(pend_tr) > 1:
                                a, g, hh = pend_tr.pop(0)
                                emit_antranspose(a, g, hh)
                        want = nu * (si * npair + step) // nstep
                        while done < want:
                            units[done]()
                            done += 1
                    prev = (at_t, qg, h, kv, an_cur[h])
                while done < nu:
                    units[done]()
                    done += 1

            # ---------------- tail ----------------
            emit_avburst(*prev)
            pend_tr.append((prev[4], prev[1], prev[2]))
            while pend_tr:
                a, g, hh = pend_tr.pop(0)
                emit_antranspose(a, g, hh)
            for tb in (12, 13, 14, 15):
                for ng in range(4):
                    emit_oproj_chunk(tb, ng, on_act=(ng % 2 == 1))

    nc.compile()
    return nc


def _prep_core_inputs(c, x, Wq, bq, Wk, bk, Wv, bv, Wo, xt_cache, fp8):
    import ml_dtypes
    bf16 = ml_dtypes.bfloat16
    g = c % 4
    b = c // 4
    f32 = np.float32
    if b not in xt_cache:
        # xt[p, c2, i, t] = x[b, t, 128*(2*c2+i)+p]; xr = 32*residual
        xm = np.ascontiguousarray(
            x[b].T.reshape(16, 128, S).reshape(8, 2, 128, S)
            .transpose(2, 0, 1, 3))
        x8 = xm.astype(fp8)
        xrr = ((xm - x8.astype(f32)) * 32.0).astype(fp8)
        xt_cache[b] = (x8, xrr)
    x8, xrr = xt_cache[b]

    def split8(wm):
        w8 = wm.astype(fp8)
        wr = ((wm - w8.astype(f32)) * 32.0).astype(fp8)
        return w8, wr

    wq_s = Wq[:, 512 * g:512 * (g + 1)].reshape(E, 8, 64)
    wq_s = wq_s[:, HEAD_PERM, :].reshape(E, 512) * f32(WS)
    wq_m = np.ascontiguousarray(
        wq_s.reshape(8, 2, 128, 4, 128).transpose(2, 3, 0, 1, 4))
    wq8, wqr = split8(wq_m)
    wk_s = Wk[:, 128 * g:128 * (g + 1)] * f32(WS)
    wk_m = np.ascontiguousarray(
        wk_s.reshape(8, 2, 128, 128).transpose(2, 0, 1, 3))
    wk8, wkr = split8(wk_m)
    wv_s = Wv[:, 128 * g:128 * (g + 1)] * f32(WS)
    wv_m = np.ascontiguousarray(
        wv_s.reshape(8, 2, 128, 128).transpose(2, 0, 1, 3))
    wv8, wvr = split8(wv_m)
    wo_s = Wo[512 * g:512 * (g + 1), :].reshape(8, 64, E)
    wo_s = wo_s[HEAD_PERM, :, :].reshape(512, E) * f32(WS)
    wo_m = np.ascontiguousarray(
        wo_s.reshape(2, 2, 128, E).transpose(2, 0, 1, 3))
    wo8 = wo_m.astype(fp8)
    dwo = (wo_m - wo8.astype(f32)).astype(fp8)
    bias = np.zeros((128, 8), f32)
    bq_s = bq[512 * g:512 * (g + 1)].reshape(8, 64)[HEAD_PERM, :].reshape(512)
    bias[:, 0:4] = bq_s.reshape(4, 128).T * WS
    bias[:, 4] = bk[128 * g:128 * (g + 1)] * WS
    bias[:, 5] = bv[128 * g:128 * (g + 1)] * WS
    bias[:, 6] = 1.0 / 1024.0
    bias[:, 7] = -3.3
    pp = np.arange(128)[:, None]
    kk = np.arange(128)[None, :]
    cmt = np.repeat(np.where(pp < kk, -240.0, 0.0)[:, None, :],
                    2, axis=1).astype(fp8)
    cmx = np.repeat(np.where(pp <= kk, -240.0, 0.0)[:, None, :],
                    2, axis=1).astype(fp8)
    seli = (np.eye(128, dtype=f32) * 240.0).astype(fp8)
    selw = np.zeros((128, 256), f32)
    selw[0, 0:128] = 240.0
    for cc in range(128, 255):
        selw[cc - 127, cc] = 240.0
    selw = selw.astype(fp8)
    wkv = np.stack([wk8, wkr, wv8, wvr], axis=1)
    msk = np.concatenate(
        [cmt.reshape(128, 256), cmx.reshape(128, 256), seli, selw],
        axis=1)
    return {"xt": x8, "xr": xrr, "wq": wq8, "wqr": wqr, "wkv": wkv,
            "wo8": wo8, "dwo": dwo, "bias": bias, "msk": msk}


def kernel(**inputs):
    import ml_dtypes
    from concourse.bass_utils import run_bass_kernel_spmd

    fp8 = ml_dtypes.float8_e4m3
    x = np.asarray(inputs["x"], np.float32)
    Wq = np.asarray(inputs["Wq"], np.float32)
    bq = np.asarray(inputs["bq"], np.float32)
    Wk = np.asarray(inputs["Wk"], np.float32)
    bk = np.asarray(inputs["bk"], np.float32)
    Wv = np.asarray(inputs["Wv"], np.float32)
    bv = np.asarray(inputs["bv"], np.float32)
    Wo = np.asarray(inputs["Wo"], np.float32)
    bo = np.asarray(inputs["bo"], np.float32)

    if "nc" not in _CACHE:
        _CACHE["nc"] = _build()
    nc = _CACHE["nc"]

    xt_cache = {}
    in_maps = [_prep_core_inputs(c, x, Wq, bq, Wk, bk, Wv, bv, Wo,
                                 xt_cache, fp8)
               for c in range(NCORE)]
    res = run_bass_kernel_spmd(nc, in_maps, list(range(NCORE)))
    parts = [res.results[c]["out"].astype(np.float32) for c in range(NCORE)]
    out0 = parts[0] + parts[1] + parts[2] + parts[3] + bo
    out1 = parts[4] + parts[5] + parts[6] + parts[7] + bo
    return np.stack([out0, out1]).astype(np.float32)


# revision 30
# speedup vs baseline: 1.0152x; 1.0104x over previous
"""GQA kernel for Trainium2, 8 NeuronCores — mixed bf16/fp8-DoubleRow.

Sharding: core c -> batch b = c//4, kv-head-group g = c%4.
Each core: 1 batch, 2 KV heads (2g, 2g+1), 8 Q heads, row-shard of W_o
(rows 512g..512g+512). Host sums the 4 partials per batch, /1024, + bo.

Precision plan (gate: rel err < 2e-2 vs abs-max):
  - Q/K/V projections: 3-term error-compensated fp8 DoubleRow,
      P = x8@W8  (psum1)   +   [(32dx)8@W8 + x8@(32dW)8]/32  (psum2),
    residuals prepped on host, merged by DVE into bf16 (error ~0.1%).
    Weights carry a x32 base scale for fp8 range; Q/K/V come out x32.
  - Scores + A@V for q-tiles 1..3 (n_eff > 512): fp8 DoubleRow from
    fp8 copies of Q/K/at/V — softmax normalization suppresses these
    errors by ~1/sqrt(n_eff).
  - Scores + A@V for q-tile 0 and the O projection: bf16 (quantization
    there is not normalization-suppressed).
  - exp bias -3.3 keeps fp8 'at' in [3e-8, 81] (e4m3 max 240).
  - Causal masks: PE matmuls with +-240 fp8 constants (2-plane product
    -115200 -> exp ~ 0).

Streams (h, kv, qg): key-block PAIRS fill a [128,2,512] psum group
(2-bank, ping-pong) -> one wide exp on ACT into at2 ring buffers.
Diagonal quads split {j0,j1} (full width) / {j2,j3} (cols 256:512,
rects memset 0).  A@V runs lagged one stream from saved at2: 4 q-subtile
slots sequentially through the work-psum ring, DVE recip+mul -> an2
(bf16), PE transpose -> attnT for the bf16 O projection.  O-proj psum
chunks DMA straight to DRAM as f32 (host unscales 1/1024).
Projection/O chunks stream between attention pairs as budgeted fills.

PSUM: 4 banks score ping-pong + 4-bank work ring.
"""

import numpy as np

E = 2048
S = 2048
B = 2
D = 64
NCORE = 8
NKB = S // 128      # 16 key blocks
WS = 32.0           # host base scale for all weights

_CACHE = {}
# tile jb holds q-heads (jb, jb+4): kv0 head dims at partitions 0:64,
# kv1 head dims at 64:128, matching the K/V partition layout
HEAD_PERM = [0, 4, 1, 5, 2, 6, 3, 7]


def _build():
    import concourse.bass as bass
    import concourse.tile as tile
    from concourse import mybir
    from concourse import bacc
    from concourse.masks import make_identity

    F32 = mybir.dt.float32
    BF16 = mybir.dt.bfloat16
    FP8 = mybir.dt.float8e4
    Exp = mybir.ActivationFunctionType.Exp
    DR = mybir.MatmulPerfMode.DoubleRow
    ADD = mybir.AluOpType.add
    MULT = mybir.AluOpType.mult

    nc = bacc.Bacc("TRN2", target_bir_lowering=False, debug=False,
                   num_devices=NCORE)

    XT = nc.declare_dram_parameter("xt", [128, 8, 2, S], FP8, isOutput=False)
    XR = nc.declare_dram_parameter("xr", [128, 8, 2, S], FP8, isOutput=False)
    WQ = nc.declare_dram_parameter("wq", [128, 4, 8, 2, 128], FP8,
                                   isOutput=False)
    WQR = nc.declare_dram_parameter("wqr", [128, 4, 8, 2, 128], FP8,
                                    isOutput=False)
    WKV = nc.declare_dram_parameter("wkv", [128, 4, 8, 2, 128], FP8,
                                    isOutput=False)
    WO8 = nc.declare_dram_parameter("wo8", [128, 2, 2, E], FP8,
                                    isOutput=False)
    DWO = nc.declare_dram_parameter("dwo", [128, 2, 2, E], FP8,
                                    isOutput=False)
    BIAS = nc.declare_dram_parameter("bias", [128, 8], F32, isOutput=False)
    MSK = nc.declare_dram_parameter("msk", [128, 896], FP8, isOutput=False)
    OUT = nc.declare_dram_parameter("out", [S, E], BF16, isOutput=True)

    with tile.TileContext(nc) as tc:
        with tc.tile_pool(name="persist", bufs=1) as persist, \
             tc.tile_pool(name="atf", bufs=2) as atf, \
             tc.tile_pool(name="atb", bufs=2) as atb, \
             tc.tile_pool(name="anp", bufs=2) as anp, \
             tc.tile_pool(name="vsp", bufs=2) as vsp, \
             tc.tile_pool(name="rlp", bufs=4) as rlp, \
             tc.tile_pool(name="tmp", bufs=2) as tmpp, \
             tc.tile_pool(name="osp", bufs=2) as osp, \
             tc.tile_pool(name="antp", bufs=4) as antp, \
             tc.tile_pool(name="scp", bufs=2, space="PSUM") as scp, \
             tc.tile_pool(name="wkp", bufs=4, space="PSUM") as wkp:

            # ---- persistent SBUF ----
            xt = persist.tile([128, 8, 2, S], FP8, tag="xt")
            xr = persist.tile([128, 8, 2, S], FP8, tag="xr")
            wq = persist.tile([128, 4, 8, 2, 128], FP8, tag="wq")
            wqr = persist.tile([128, 4, 8, 2, 128], FP8, tag="wqr")
            wkv = persist.tile([128, 4, 8, 2, 128], FP8, tag="wkv")
            wk, wkr, wv, wvr = (wkv[:, 0], wkv[:, 1], wkv[:, 2],
                                wkv[:, 3])
            wo8 = persist.tile([128, 2, 2, E], FP8, tag="wo8")
            dwo = persist.tile([128, 2, 2, E], FP8, tag="dwo")
            qtb = [persist.tile([128, S], BF16, tag=f"qtb{j}", name=f"qtb{j}")
                   for j in range(4)]
            qt8 = [persist.tile([128, S], FP8, tag=f"qt8{j}", name=f"qt8{j}")
                   for j in range(4)]
            ktb = persist.tile([128, S], BF16, tag="ktb")
            kt8 = persist.tile([128, 2, S], FP8, tag="kt8")
            vtb = persist.tile([128, 2, NKB, D + 1], BF16, tag="vtb")
            vt8 = persist.tile([128, 2, NKB, D + 1], FP8, tag="vt8")
            attnT8 = [persist.tile([128, 2, S], FP8, tag=f"attnT8{j}",
                                   name=f"attnT8{j}") for j in range(2)]
            datT8 = [persist.tile([128, 2, S], FP8, tag=f"datT8{j}",
                                  name=f"datT8{j}") for j in range(2)]
            msk = persist.tile([128, 896], FP8, tag="msk")
            cmt = msk[:, 0:256].rearrange("p (a b) -> p a b", a=2)
            cmx = msk[:, 256:512].rearrange("p (a b) -> p a b", a=2)
            seli = msk[:, 512:640]
            selw = msk[:, 640:896]
            id128 = persist.tile([128, 128], BF16, tag="id128")
            id64 = persist.tile([128, D], BF16, tag="id64")
            bias_t = persist.tile([128, 8], F32, tag="bias")

            # ---- input DMAs, ordered so first streams start early ----
            nc.sync.dma_start(out=wq[:, 0], in_=WQ[:, 0])
            nc.sync.dma_start(out=xt[:, :, :, 0:512], in_=XT[:, :, :, 0:512])
            nc.sync.dma_start(out=wkv[:, 0:2], in_=WKV[:, 0:2])
            nc.sync.dma_start(out=xr[:, :, :, 0:512], in_=XR[:, :, :, 0:512])
            nc.sync.dma_start(out=wqr[:, 0], in_=WQR[:, 0])
            nc.sync.dma_start(out=bias_t, in_=BIAS[:, :])
            nc.sync.dma_start(out=msk, in_=MSK[:, :])
            nc.sync.dma_start(out=wkv[:, 2:4], in_=WKV[:, 2:4])
            nc.sync.dma_start(out=wq[:, 1], in_=WQ[:, 1])
            nc.sync.dma_start(out=wqr[:, 1], in_=WQR[:, 1])
            nc.sync.dma_start(out=xt[:, :, :, 512:1024],
                              in_=XT[:, :, :, 512:1024])
            nc.sync.dma_start(out=xr[:, :, :, 512:1024],
                              in_=XR[:, :, :, 512:1024])
            nc.sync.dma_start(out=wq[:, 2], in_=WQ[:, 2])
            nc.sync.dma_start(out=wq[:, 3], in_=WQ[:, 3])
            nc.sync.dma_start(out=wqr[:, 2], in_=WQR[:, 2])
            nc.sync.dma_start(out=wqr[:, 3], in_=WQR[:, 3])
            nc.sync.dma_start(out=xt[:, :, :, 1024:1536],
                              in_=XT[:, :, :, 1024:1536])
            nc.sync.dma_start(out=xt[:, :, :, 1536:2048],
                              in_=XT[:, :, :, 1536:2048])
            nc.sync.dma_start(out=xr[:, :, :, 1024:1536],
                              in_=XR[:, :, :, 1024:1536])
            nc.sync.dma_start(out=xr[:, :, :, 1536:2048],
                              in_=XR[:, :, :, 1536:2048])
            nc.sync.dma_start(out=wo8, in_=WO8[:, :])
            nc.sync.dma_start(out=dwo, in_=DWO[:, :])

            make_identity(nc, id128)
            make_identity(nc, id64[0:64, :])
            make_identity(nc, id64[64:128, :])
            nc.gpsimd.memset(kt8[:, 1, :], 0.0)        # zero K plane
            nc.gpsimd.memset(vtb[:, :, :, D:D + 1], 1.0)   # denom ones
            nc.gpsimd.memset(vt8[:, :, :, D:D + 1], 1.0)

            # ---------------- comp3 projection fills ----------------
            def comp3(st, w8, wr, cols, n):
                """3-term compensated projection as 3 PE sub-units."""
                def u1():
                    st["ps1"] = wkp.tile([128, n], F32, tag="wk", name="ps1")
                    for c2 in range(8):
                        nc.tensor.matmul(
                            st["ps1"], w8[:, c2], xt[:, c2, :, cols],
                            start=(c2 == 0), stop=(c2 == 7),
                            perf_mode=DR, skip_group_check=True)

                def u2():
                    st["ps2"] = wkp.tile([128, n], F32, tag="wk", name="ps2")
                    for c2 in range(8):
                        nc.tensor.matmul(
                            st["ps2"], w8[:, c2], xr[:, c2, :, cols],
                            start=(c2 == 0), stop=False,
                            perf_mode=DR, skip_group_check=True)

                def u3():
                    for c2 in range(8):
                        nc.tensor.matmul(
                            st["ps2"], wr[:, c2], xt[:, c2, :, cols],
                            start=False, stop=(c2 == 7),
                            perf_mode=DR, skip_group_check=True)
                return [u1, u2, u3]

            def merge(st, out_b, bcol, n):
                """DVE: out_b = ps1 + ps2/32 + bias."""
                tm = tmpp.tile([128, 512], F32, tag="tm", name="tm")
                nc.vector.tensor_scalar(
                    tm[:, 0:n], st["ps2"], 1.0 / 32.0,
                    bias_t[:, bcol:bcol + 1], MULT, ADD)
                nc.vector.tensor_add(out_b, st["ps1"], tm[:, 0:n])

            def make_qproj(t5, jb):
                cols = bass.ds(t5 * 512, 512)
                st = {}

                def seg(which, c0, alloc=False, start=False, stop=False):
                    def f():
                        if alloc:
                            st[which] = wkp.tile([128, 512], F32, tag="wk",
                                                 name=which)
                        w = wq[:, jb] if which == "ps1" or c0 >= 100                             else wq[:, jb]
                        for c2 in range(c0 % 100, c0 % 100 + 4):
                            if which == "ps1":
                                lhs, rhs = wq[:, jb, c2], xt[:, c2, :, cols]
                            elif c0 < 100:
                                lhs, rhs = wq[:, jb, c2], xr[:, c2, :, cols]
                            else:
                                lhs, rhs = wqr[:, jb, c2], xt[:, c2, :, cols]
                            nc.tensor.matmul(
                                st[which], lhs, rhs,
                                start=(start and c2 == c0 % 100),
                                stop=(stop and c2 == c0 % 100 + 3),
                                perf_mode=DR, skip_group_check=True)
                    return f

                def fin():
                    seg("ps2", 104, stop=True)()
                    merge(st, qtb[jb][:, cols], jb, 512)
                    nc.gpsimd.tensor_copy(qt8[jb][:, cols], qtb[jb][:, cols])
                return [seg("ps1", 0, alloc=True, start=True),
                        seg("ps1", 4, stop=True),
                        seg("ps2", 0, alloc=True, start=True),
                        seg("ps2", 4),
                        seg("ps2", 100),
                        fin]

            def make_kproj(tg):
                cols = bass.ds(tg * 256, 256)
                st = {}
                us = comp3(st, wk, wkr, cols, 256)

                def fin():
                    us[2]()
                    merge(st, ktb[:, cols], 4, 256)
                    nc.gpsimd.tensor_copy(kt8[:, 0, cols], ktb[:, cols])
                return [us[0], us[1], fin]

            def make_vproj(tg):
                cols = bass.ds(tg * 256, 256)
                st = {}
                us = comp3(st, wv, wvr, cols, 256)

                def fin():
                    us[2]()
                    st["vs"] = vsp.tile([128, 256], BF16, tag="vs",
                                        name="vs")
                    merge(st, st["vs"], 5, 256)

                def transp(kv):
                    def f():
                        tp = wkp.tile([128, 2, D], BF16, tag="wk",
                                      name="vtp")
                        for tc2 in range(2):
                            nc.tensor.transpose(
                                tp[:, tc2, :],
                                st["vs"][kv * 64:kv * 64 + 64,
                                         tc2 * 128:(tc2 + 1) * 128],
                                id64[kv * 64:kv * 64 + 64, :])
                        nc.vector.tensor_copy(
                            vtb[:, kv, 2 * tg:2 * tg + 2, 0:D], tp)
                        nc.gpsimd.tensor_copy(
                            vt8[:, kv, 2 * tg:2 * tg + 2, 0:D],
                            vtb[:, kv, 2 * tg:2 * tg + 2, 0:D])
                    return f
                return [us[0], us[1], fin, transp(0), transp(1)]

            osp_tiles = {}
            Copy = mybir.ActivationFunctionType.Copy

            def emit_oproj_chunk(tb, ng, on_act=False):
                if tb not in osp_tiles:
                    osp_tiles[tb] = osp.tile([128, E], BF16, tag="os",
                                             name="ostage")
                op = wkp.tile([128, 512], F32, tag="wk", name="opc")
                tc_ = bass.ds(tb * 128, 128)
                ngc = bass.ds(ng * 512, 512)
                for ti, (lh, rh) in enumerate(
                        [(attnT8, wo8), (datT8, wo8), (attnT8, dwo)]):
                    for jbp in range(2):
                        nc.tensor.matmul(
                            op, lh[jbp][:, :, tc_], rh[:, jbp, :, ngc],
                            start=(ti == 0 and jbp == 0),
                            stop=(ti == 2 and jbp == 1),
                            perf_mode=DR, skip_group_check=True)
                if on_act:
                    nc.scalar.activation(
                        osp_tiles[tb][:, ng * 512:(ng + 1) * 512], op,
                        Copy, bias=0.0, scale=bias_t[:, 6:7])
                else:
                    nc.vector.tensor_scalar_mul(
                        osp_tiles[tb][:, ng * 512:(ng + 1) * 512], op,
                        bias_t[:, 6:7])
                nc.sync.dma_start(
                    out=OUT[tb * 128:(tb + 1) * 128,
                            ng * 512:(ng + 1) * 512],
                    in_=osp_tiles[tb][:, ng * 512:(ng + 1) * 512])

            def qu(t5, jb):
                return make_qproj(t5, jb)

            def kvu(tg):
                return make_kproj(tg) + make_vproj(tg)

            op_st = {}

            def emit_oproj_half(tb, ng, half):
                key = (tb, ng)
                if half == 0:
                    if tb not in osp_tiles:
                        osp_tiles[tb] = osp.tile([128, E], BF16, tag="os",
                                                 name="ostage")
                    op_st[key] = wkp.tile([128, 512], F32, tag="wk",
                                          name="opc")
                op = op_st[key]
                tc_ = bass.ds(tb * 128, 128)
                ngc = bass.ds(ng * 512, 512)
                terms = [(attnT8, wo8, 0), (attnT8, wo8, 1),
                         (datT8, wo8, 0), (datT8, wo8, 1),
                         (attnT8, dwo, 0), (attnT8, dwo, 1)]
                rng = terms[0:3] if half == 0 else terms[3:6]
                for ti, (lh, rh, jbp) in enumerate(rng):
                    nc.tensor.matmul(
                        op, lh[jbp][:, :, tc_], rh[:, jbp, :, ngc],
                        start=(half == 0 and ti == 0),
                        stop=(half == 1 and ti == 2),
                        perf_mode=DR, skip_group_check=True)
                if half == 1:
                    nc.vector.tensor_scalar_mul(
                        osp_tiles[tb][:, ng * 512:(ng + 1) * 512], op,
                        bias_t[:, 6:7])
                    nc.sync.dma_start(
                        out=OUT[tb * 128:(tb + 1) * 128,
                                ng * 512:(ng + 1) * 512],
                        in_=osp_tiles[tb][:, ng * 512:(ng + 1) * 512])
                    del op_st[key]

            def ou(tbs):
                return [(lambda tb=tb, ng=ng, hf=hf:
                         emit_oproj_half(tb, ng, hf))
                        for tb in tbs for ng in range(4) for hf in (0, 1)]

            # ---------------- attention streams ----------------
            def emit_stream_scores(at_t, qg, h, kv):
                nkb = 4 * qg + 4
                npair = nkb // 2
                qoff = kv * 64
                q0 = qg * 512

                def score8(bank_ap, kb, c0, start, stop):
                    n = 512 - c0
                    nc.tensor.matmul(
                        bank_ap[:, c0:512],
                        kt8[qoff:qoff + 64, :, kb * 128:(kb + 1) * 128],
                        qt8[h][qoff:qoff + 64, q0 + c0:q0 + 512]
                        .unsqueeze(1).broadcast_to([64, 2, n]),
                        start=start, stop=stop,
                        perf_mode=DR, skip_group_check=True)

                def scoreb(bank_ap, kb, c0, start, stop):
                    nc.tensor.matmul(
                        bank_ap[:, c0:512],
                        ktb[qoff:qoff + 64, kb * 128:(kb + 1) * 128],
                        qtb[h][qoff:qoff + 64, q0 + c0:q0 + 512],
                        start=start, stop=stop, skip_group_check=True)

                score = scoreb if qg == 0 else score8

                def mask_tril(bank_ap, c0):
                    nc.tensor.matmul(
                        bank_ap[:, c0:c0 + 128], cmt,
                        seli.unsqueeze(1).broadcast_to([128, 2, 128]),
                        start=True, stop=False,
                        perf_mode=DR, skip_group_check=True)

                for pr in range(npair):
                    kb0 = 2 * pr
                    di = pr - (npair - 2)  # 0 => {j0,j1}, 1 => {j2,j3}
                    sc = scp.tile([128, 2, 512], F32, tag="sc", name="sc")
                    if di < 0:
                        for b in range(2):
                            score(sc[:, b], kb0 + b, 0, True, True)
                    elif di == 0:
                        mask_tril(sc[:, 0], 0)
                        score(sc[:, 0], kb0, 0, False, True)
                        nc.tensor.matmul(
                            sc[:, 1, 0:256], cmx,
                            selw.unsqueeze(1).broadcast_to([128, 2, 256]),
                            start=True, stop=False,
                            perf_mode=DR, skip_group_check=True)
                        score(sc[:, 1], kb0 + 1, 128, False, True)
                    else:
                        mask_tril(sc[:, 0], 256)
                        score(sc[:, 0], kb0, 256, False, True)
                        # j3: cols [256:384) fully masked + tril [384:512)
                        nc.tensor.matmul(
                            sc[:, 1, 256:512], cmx,
                            selw.unsqueeze(1).broadcast_to([128, 2, 256]),
                            start=True, stop=False,
                            perf_mode=DR, skip_group_check=True)
                        score(sc[:, 1], kb0 + 1, 384, False, True)
                    if di == 1:
                        nc.scalar.activation(
                            at_t[:, kb0:kb0 + 2, 256:512],
                            sc[:, :, 256:512], Exp,
                            bias=bias_t[:, 7:8], scale=1.0 / 8192.0)
                    else:
                        nc.scalar.activation(
                            at_t[:, kb0:kb0 + 2, :], sc, Exp,
                            bias=bias_t[:, 7:8], scale=1.0 / 8192.0)
                    yield

            def emit_avburst(at_t, qg, h, kv, an_t):
                nkb = 4 * qg + 4
                npair = nkb // 2
                for s in range(4):
                    slot = wkp.tile([128, D + 1], F32, tag="wk", name="av")
                    if qg == 0:
                        for kb in range(nkb):
                            nc.tensor.matmul(
                                slot,
                                at_t[:, kb, s * 128:(s + 1) * 128],
                                vtb[:, kv, kb, 0:D + 1],
                                start=(kb == 0), stop=(kb == nkb - 1),
                                skip_group_check=True)
                    else:
                        for pr in range(npair):
                            nc.tensor.matmul(
                                slot,
                                at_t[:, 2 * pr:2 * pr + 2,
                                     s * 128:(s + 1) * 128],
                                vt8[:, kv, 2 * pr:2 * pr + 2, 0:D + 1],
                                start=(pr == 0), stop=(pr == npair - 1),
                                perf_mode=DR, skip_group_check=True)
                    rl = rlp.tile([128, 1], F32, tag="rl", name="rl")
                    nc.vector.reciprocal_approx_fast(rl, slot[:, D:D + 1])
                    nc.vector.tensor_scalar_mul(
                        an_t[:, s, kv * 64:kv * 64 + 64], slot[:, 0:D],
                        rl[:, 0:1])

            def emit_antranspose(an_t, qg, h):
                q0 = qg * 512
                jbp, pl = h // 2, h % 2
                for qb in range(4):
                    tp = wkp.tile([128, 128], BF16, tag="wk", name="atp2")
                    nc.tensor.transpose(tp, an_t[:, qb, :], id128)
                    cols = bass.ds(q0 + qb * 128, 128)
                    ab = antp.tile([128, 128], BF16, tag="ab", name="ab")
                    nc.vector.tensor_copy(ab, tp)
                    nc.gpsimd.tensor_copy(attnT8[jbp][:, pl, cols], ab)
                    nc.gpsimd.tensor_sub(datT8[jbp][:, pl, cols], ab,
                                         attnT8[jbp][:, pl, cols])

            # ---------------- phase 1: first-stream prereqs ----------------
            for f in qu(0, 0) + kvu(0) + kvu(1):
                f()

            fills = {
                0: qu(0, 1) + qu(0, 2) + qu(0, 3) + qu(1, 0) + qu(1, 1) +
                   kvu(2) + kvu(3),
                1: qu(1, 2) + qu(1, 3) + qu(2, 0) + qu(2, 1) +
                   kvu(4) + kvu(5),
                2: qu(2, 2) + qu(2, 3) + qu(3, 0) + qu(3, 1) +
                   kvu(6) + kvu(7) + ou((0, 1)),
                3: qu(3, 2) + qu(3, 3) + ou((2, 3)) + ou((4, 5)) +
                   ou((6, 7)) + ou((8, 9, 10, 11)),
            }

            # ---------------- main stream loop ----------------
            prev = None          # (at_t, qg, h, kv, an_t)
            an_cur = {}
            pend_tr = []

            for qg in range(4):
                units = list(fills[qg])
                nu = len(units)
                done = 0
                npair = 2 * qg + 2
                nstep = 8 * npair
                for si, (h, kv) in enumerate([(h, kv) for h in range(4)
                                              for kv in range(2)]):
                    nkb = 4 * qg + 4
                    if qg == 0:
                        at_t = atb.tile([128, 4, 512], BF16, tag="atb",
                                        name="atb")
                    else:
                        at_t = atf.tile([128, NKB, 512], FP8, tag="atf",
                                        name="atf")
                    # zero never-exp'd rects of the {j2,j3} diag pair
                    nc.gpsimd.memset(at_t[:, nkb - 2:nkb, 0:256], 0.0)
                    if kv == 0:
                        an_cur[h] = anp.tile([128, 4, 128], BF16, tag="an",
                                             name="an2")
                    gen = emit_stream_scores(at_t, qg, h, kv)
                    step = 0
                    for _ in gen:
                        step += 1
                        # lagged work tucked behind this stream's first exps
                        if step == 1 and prev is not None:
                            emit_avburst(*prev)
                            if prev[3] == 1:
                                pend_tr.append((prev[4], prev[1], prev[2]))
                            prev = None
                        if step == 2:
                            # last stream: drain fully so only h3's
                            # transposes remain in the serial tail
                            lim = 0 if (qg == 3 and si == 7) else 1
                            while len(pend_tr) > lim:
                                a, g, hh = pend_tr.pop(0)
                                emit_antranspose(a, g, hh)
                        want = nu * (si * npair + step) // nstep
                        while done < want:
                            units[done]()
                            done += 1
                    prev = (at_t, qg, h, kv, an_cur[h])
                while done < nu:
                    units[done]()
                    done += 1

            # ---------------- tail ----------------
            emit_avburst(*prev)
            pend_tr.append((prev[4], prev[1], prev[2]))
            while pend_tr:
                a, g, hh = pend_tr.pop(0)
                emit_antranspose(a, g, hh)
            for tb in (12, 13, 14, 15):
                for ng in range(4):
                    emit_oproj_chunk(tb, ng, on_act=(ng % 2 == 1))

    nc.compile()
    return nc


def _prep_core_inputs(c, x, Wq, bq, Wk, bk, Wv, bv, Wo, xt_cache, fp8):
    import ml_dtypes
    bf16 = ml_dtypes.bfloat16
    g = c % 4
    b = c // 4
    f32 = np.float32
    if b not in xt_cache:
        # xt[p, c2, i, t] = x[b, t, 128*(2*c2+i)+p]; xr = 32*residual
        xm = np.ascontiguousarray(
            x[b].T.reshape(16, 128, S).reshape(8, 2, 128, S)
            .transpose(2, 0, 1, 3))
        x8 = xm.astype(fp8)
        xrr = ((xm - x8.astype(f32)) * 32.0).astype(fp8)
        xt_cache[b] = (x8, xrr)
    x8, xrr = xt_cache[b]

    def split8(wm):
        w8 = wm.astype(fp8)
        wr = ((wm - w8.astype(f32)) * 32.0).astype(fp8)
        return w8, wr

    wq_s = Wq[:, 512 * g:512 * (g + 1)].reshape(E, 8, 64)
    wq_s = wq_s[:, HEAD_PERM, :].reshape(E, 512) * f32(WS)
    wq_m = np.ascontiguousarray(
        wq_s.reshape(8, 2, 128, 4, 128).transpose(2, 3, 0, 1, 4))
    wq8, wqr = split8(wq_m)
    wk_s = Wk[:, 128 * g:128 * (g + 1)] * f32(WS)
    wk_m = np.ascontiguousarray(
        wk_s.reshape(8, 2, 128, 128).transpose(2, 0, 1, 3))
    wk8, wkr = split8(wk_m)
    wv_s = Wv[:, 128 * g:128 * (g + 1)] * f32(WS)
    wv_m = np.ascontiguousarray(
        wv_s.reshape(8, 2, 128, 128).transpose(2, 0, 1, 3))
    wv8, wvr = split8(wv_m)
    wo_s = Wo[512 * g:512 * (g + 1), :].reshape(8, 64, E)
    wo_s = wo_s[HEAD_PERM, :, :].reshape(512, E) * f32(WS)
    wo_m = np.ascontiguousarray(
        wo_s.reshape(2, 2, 128, E).transpose(2, 0, 1, 3))
    wo8 = wo_m.astype(fp8)
    dwo = (wo_m - wo8.astype(f32)).astype(fp8)
    bias = np.zeros((128, 8), f32)
    bq_s = bq[512 * g:512 * (g + 1)].reshape(8, 64)[HEAD_PERM, :].reshape(512)
    bias[:, 0:4] = bq_s.reshape(4, 128).T * WS
    bias[:, 4] = bk[128 * g:128 * (g + 1)] * WS
    bias[:, 5] = bv[128 * g:128 * (g + 1)] * WS
    bias[:, 6] = 1.0 / 1024.0
    bias[:, 7] = -3.3
    pp = np.arange(128)[:, None]
    kk = np.arange(128)[None, :]
    cmt = np.repeat(np.where(pp < kk, -240.0, 0.0)[:, None, :],
                    2, axis=1).astype(fp8)
    cmx = np.repeat(np.where(pp <= kk, -240.0, 0.0)[:, None, :],
                    2, axis=1).astype(fp8)
    seli = (np.eye(128, dtype=f32) * 240.0).astype(fp8)
    selw = np.zeros((128, 256), f32)
    selw[0, 0:128] = 240.0
    for cc in range(128, 255):
        selw[cc - 127, cc] = 240.0
    selw = selw.astype(fp8)
    wkv = np.stack([wk8, wkr, wv8, wvr], axis=1)
    msk = np.concatenate(
        [cmt.reshape(128, 256), cmx.reshape(128, 256), seli, selw],
        axis=1)
    return {"xt": x8, "xr": xrr, "wq": wq8, "wqr": wqr, "wkv": wkv,
            "wo8": wo8, "dwo": dwo, "bias": bias, "msk": msk}


def kernel(**inputs):
    import ml_dtypes
    from concourse.bass_utils import run_bass_kernel_spmd

    fp8 = ml_dtypes.float8_e4m3
    x = np.asarray(inputs["x"], np.float32)
    Wq = np.asarray(inputs["Wq"], np.float32)
    bq = np.asarray(inputs["bq"], np.float32)
    Wk = np.asarray(inputs["Wk"], np.float32)
    bk = np.asarray(inputs["bk"], np.float32)
    Wv = np.asarray(inputs["Wv"], np.float32)
    bv = np.asarray(inputs["bv"], np.float32)
    Wo = np.asarray(inputs["Wo"], np.float32)
    bo = np.asarray(inputs["bo"], np.float32)

    if "nc" not in _CACHE:
        _CACHE["nc"] = _build()
    nc = _CACHE["nc"]

    xt_cache = {}
    in_maps = [_prep_core_inputs(c, x, Wq, bq, Wk, bk, Wv, bv, Wo,
                                 xt_cache, fp8)
               for c in range(NCORE)]
    res = run_bass_kernel_spmd(nc, in_maps, list(range(NCORE)))
    parts = [res.results[c]["out"].astype(np.float32) for c in range(NCORE)]
    out0 = parts[0] + parts[1] + parts[2] + parts[3] + bo
    out1 = parts[4] + parts[5] + parts[6] + parts[7] + bo
    return np.stack([out0, out1]).astype(np.float32)
